# revision 11
# baseline (speedup 1.0000x reference)
"""Trainium2 Bass kernel for nn_Decoder (LSTMCell -> GRUCell -> Linear decode).

Strategy (8 NeuronCores, one chip):
  - Hidden dim H=2048 sharded 8 ways (256/core). Each core holds the weight
    rows for its hidden slice of the LSTM/GRU gates in SBUF (bf16), computes
    its gate shard with batch-major col-tiled matmuls (stationary = h-major
    state tiles [128,32], moving = weight columns), applies the elementwise
    cell updates in fp32, stream-transposes its new state shard to h-major
    bf16 and exchanges it via two pipelined AllGathers per step: AG_hc
    carries [h(s+1), c(s+1)] (issued right after the LSTM, while the GRU
    still computes) and AG_hg carries [hg(s)].
  - Hidden-index layout l = 128*c2 + 32*jp + i is chosen so the 32x32-block
    StreamTranspose of the [128,64] state tile directly yields the h-major
    shard, and every DRAM exchange is a single strided DMA.
  - The vocab-sharded linear decode (1000 rows/core) consumes a double-
    buffered history of gathered hg, one vocab tile per step, hiding under
    the collective latency and keeping TensorE warm.
  - kernel(**inputs) takes FULL inputs, shards on host, runs the SPMD NEFF
    on cores 0-7 via run_bass_kernel_spmd, reassembles the FULL output.
"""
import os
import sys

import numpy as np

for _p in ("/root/.axon_site", "/root/.axon_site/_ro/trn_rl_repo",
           "/root/.axon_site/_ro/pypackages", "/opt/trn_rl_repo"):
    if os.path.isdir(_p) and _p not in sys.path:
        sys.path.append(_p)

import concourse.bacc as bacc
import concourse.bass as bass
import concourse.mybir as mybir
import concourse.tile as tile
from concourse import bass_utils

import ml_dtypes

BF16 = ml_dtypes.bfloat16
F32 = mybir.dt.float32
BF = mybir.dt.bfloat16
AF = mybir.ActivationFunctionType

NC = 8          # cores
B = 32          # batch
T = 40          # caption length
TS = T - 1      # recurrent steps
V = 8000
E = 50
H = 2048
HS = H // NC    # 256 hidden per core
VS = V // NC    # 1000 vocab per core
KT = H // 128   # 16 contraction tiles
VT = 8          # vocab tiles per core
VTW = VS // VT  # 125 cols per vocab tile
GL = 4 * HS     # 1024 lstm gate cols per core
GG = 3 * HS     # 768 gru gate cols per matmul per core
NHIST = 8       # decode chunk length

_BUILD_CACHE = {}


def _build(ts=TS):
    nc = bacc.Bacc("TRN2", target_bir_lowering=False, debug=False,
                   enable_asserts=True, num_devices=NC)

    # ---- external I/O (per core) ----
    wl_in = nc.dram_tensor("wl", [H, GL], BF, kind="ExternalInput")
    wih_in = nc.dram_tensor("wih", [E + 1, GL], BF, kind="ExternalInput")
    wgi_in = nc.dram_tensor("wgi", [H, GG], BF, kind="ExternalInput")
    brgi_in = nc.dram_tensor("brgi", [1, GG], BF, kind="ExternalInput")
    wgh_in = nc.dram_tensor("wgh", [H, GG], BF, kind="ExternalInput")
    brgh_in = nc.dram_tensor("brgh", [1, GG], BF, kind="ExternalInput")
    linw_in = nc.dram_tensor("linw", [H, VS], BF, kind="ExternalInput")
    linb_in = nc.dram_tensor("linb", [VTW, VT], F32, kind="ExternalInput")
    xs_in = nc.dram_tensor("xs_aug", [E + 1, ts * B], BF, kind="ExternalInput")
    featT_in = nc.dram_tensor("featT", [H, B], BF, kind="ExternalInput")
    featsh_in = nc.dram_tensor("feat_sh", [HS, B], BF, kind="ExternalInput")
    featblk_in = nc.dram_tensor("feat_blk", [128, 64], F32, kind="ExternalInput")
    ones_in = nc.dram_tensor("ones", [1, B], BF, kind="ExternalInput")
    out = nc.dram_tensor("out", [VS, ts * B], F32, kind="ExternalOutput")

    with tile.TileContext(nc) as tc:
        with (
            tc.tile_pool(name="const", bufs=1) as cpool,
            tc.tile_pool(name="stat", bufs=3) as spool,
            tc.tile_pool(name="state", bufs=2) as stpool,
            tc.tile_pool(name="ew", bufs=3) as ew,
            tc.tile_pool(name="psl", bufs=2, space="PSUM") as psl,
            tc.tile_pool(name="psg", bufs=2, space="PSUM") as psg,
            tc.tile_pool(name="psd", bufs=2, space="PSUM") as psd,
        ):
            # ---- load weights / constants into SBUF (single strided DMAs) --
            wl_sb = cpool.tile([128, KT * GL], BF)
            nc.sync.dma_start(
                wl_sb[:, :].rearrange("r (k c) -> r k c", k=KT),
                wl_in[:, :].rearrange("(k r) c -> r k c", k=KT))
            wgi_sb = cpool.tile([128, KT * GG], BF)
            nc.sync.dma_start(
                wgi_sb[:, :].rearrange("r (k c) -> r k c", k=KT),
                wgi_in[:, :].rearrange("(k r) c -> r k c", k=KT))
            wgh_sb = cpool.tile([128, KT * GG], BF)
            nc.sync.dma_start(
                wgh_sb[:, :].rearrange("r (k c) -> r k c", k=KT),
                wgh_in[:, :].rearrange("(k r) c -> r k c", k=KT))
            linw_sb = cpool.tile([128, KT * VS], BF)
            nc.scalar.dma_start(
                linw_sb[:, :].rearrange("r (k c) -> r k c", k=KT),
                linw_in[:, :].rearrange("(k r) c -> r k c", k=KT))
            wih_sb = cpool.tile([E + 1, GL], BF)
            nc.sync.dma_start(wih_sb[:, :], wih_in[:, :])
            brgi_sb = cpool.tile([1, GG], BF)
            nc.sync.dma_start(brgi_sb[:, :], brgi_in[:, :])
            brgh_sb = cpool.tile([1, GG], BF)
            nc.sync.dma_start(brgh_sb[:, :], brgh_in[:, :])
            linb_sb = cpool.tile([VTW, VT], F32)
            nc.scalar.dma_start(linb_sb[:, :], linb_in[:, :])
            xs_sb = cpool.tile([E + 1, ts * B], BF)
            nc.sync.dma_start(xs_sb[:, :], xs_in[:, :])
            ones_sb = cpool.tile([1, B], BF)
            nc.sync.dma_start(ones_sb[:, :], ones_in[:, :])
            feat_blk = cpool.tile([128, 64], F32)
            nc.sync.dma_start(feat_blk[:, :], featblk_in[:, :])
            hT_init = cpool.tile([128, KT * B], BF)
            nc.sync.dma_start(
                hT_init[:, :].rearrange("r (k b) -> r k b", k=KT),
                featT_in[:, :].rearrange("(k r) b -> r k b", k=KT))
            # double-buffered gathered-hg history (h-major, bf16)
            hists = [cpool.tile([128, KT * NHIST * B], BF, name=f"hist{p}")
                     for p in range(2)]

            def hist_slot_half(m, half):
                """DMA-dst AP (r, k8, b) for hg(m)'s history slot, k%2==half."""
                h = hists[(m // NHIST) % 2]
                return h[:, :].rearrange(
                    "r (k8 k2 t b) -> k2 t r k8 b", k8=NC, k2=2,
                    t=NHIST)[half][m % NHIST]

            def hist_slot_k(m, k):
                """Stationary AP [128, B] for hg(m) k-tile k."""
                h = hists[(m // NHIST) % 2]
                return h[:, k * NHIST * B + (m % NHIST) * B:
                         k * NHIST * B + (m % NHIST) * B + B]

            def emit_lstm(step, hT_ap, c_prev, y):
                gsum = psl.tile([128, 256], F32, tag="ps_l", name=f"ps_l{step}")
                for k in range(KT):
                    for j in range(4):
                        nc.tensor.matmul(
                            gsum[32 * j:32 * j + 32, :],
                            hT_ap(k),
                            wl_sb[:, k * GL + j * 256:k * GL + j * 256 + 256],
                            start=(k == 0), stop=False,
                            tile_position=(0, 32 * j),
                        )
                for j in range(4):
                    nc.tensor.matmul(
                        gsum[32 * j:32 * j + 32, :],
                        xs_sb[:, step * B:(step + 1) * B],
                        wih_sb[:, j * 256:j * 256 + 256],
                        start=False, stop=True,
                        tile_position=(0, 32 * j),
                    )
                tc2 = tc
                with tc2.high_priority():
                    return emit_lstm_tail(step, gsum, c_prev, y)

            def emit_lstm_tail(step, gsum, c_prev, y):
                tg = ew.tile([128, 64], F32, tag="tg", name=f"tg{step}")
                nc.scalar.activation(tg[:, :], gsum[:, 128:192], AF.Tanh)
                sif = ew.tile([128, 128], F32, tag="sif", name=f"sif{step}")
                nc.scalar.activation(sif[:, :], gsum[:, 0:128], AF.Sigmoid)
                so = ew.tile([128, 64], F32, tag="so", name=f"so{step}")
                nc.scalar.activation(so[:, :], gsum[:, 192:256], AF.Sigmoid)
                t1 = ew.tile([128, 64], F32, tag="t1", name=f"t1_{step}")
                nc.vector.tensor_mul(t1[:, :], sif[:, 0:64], tg[:, :])
                cn = stpool.tile([128, 64], F32, tag="c_st", name=f"c_st{step}")
                nc.vector.tensor_mul(cn[:, :], sif[:, 64:128], c_prev[:, :])
                nc.vector.tensor_add(cn[:, :], cn[:, :], t1[:, :])
                cb = ew.tile([128, 64], BF, tag="cb", name=f"cb{step}")
                nc.vector.tensor_copy(cb[:, :], cn[:, :])
                nc.vector.transpose(y[:, 64:128], cb[:, :])
                tc_ = ew.tile([128, 64], F32, tag="tc", name=f"tc{step}")
                nc.scalar.activation(tc_[:, :], cn[:, :], AF.Tanh)
                hb = ew.tile([128, 64], BF, tag="hb", name=f"hb{step}")
                nc.vector.tensor_mul(hb[:, :], so[:, :], tc_[:, :])
                nc.vector.transpose(y[:, 0:64], hb[:, :])
                return cn

            def emit_gru_gi(step, cT_st):
                """gi matmul (+bias via ones-row) — depends only on AG_hc."""
                pgi = psg.tile([128, 192], F32, tag="ps_gi", name=f"ps_gi{step}")
                for k in range(KT):
                    for j in range(4):
                        nc.tensor.matmul(
                            pgi[32 * j:32 * j + 32, :],
                            cT_st[:, k * B:(k + 1) * B],
                            wgi_sb[:, k * GG + j * 192:k * GG + j * 192 + 192],
                            start=(k == 0), stop=False,
                            tile_position=(0, 32 * j),
                        )
                for j in range(4):
                    nc.tensor.matmul(
                        pgi[32 * j:32 * j + 32, :],
                        ones_sb[:, :],
                        brgi_sb[:, j * 192:(j + 1) * 192],
                        start=False, stop=(j == 3),
                        tile_position=(0, 32 * j),
                    )
                gi_sb = ew.tile([128, 192], F32, tag="gi_sb",
                                name=f"gi_sb{step}")
                nc.vector.tensor_copy(gi_sb[:, :], pgi[:, :])
                return gi_sb

            def emit_gru_gh(step, pgi, hg_prev):
                """gh matmul + elementwise tail (the latency-critical part)."""
                pgh = psg.tile([128, 192], F32, tag="ps_gh", name=f"ps_gh{step}")
                for k in range(KT):
                    for j in range(4):
                        nc.tensor.matmul(
                            pgh[32 * j:32 * j + 32, :],
                            hist_slot_k(step - 1, k),
                            wgh_sb[:, k * GG + j * 192:k * GG + j * 192 + 192],
                            start=(k == 0), stop=False,
                            tile_position=(0, 32 * j),
                        )
                for j in range(4):
                    nc.tensor.matmul(
                        pgh[32 * j:32 * j + 32, :],
                        ones_sb[:, :],
                        brgh_sb[:, j * 192:(j + 1) * 192],
                        start=False, stop=(j == 3),
                        tile_position=(0, 32 * j),
                    )
                with tc.high_priority():
                    trz = ew.tile([128, 128], F32, tag="trz", name=f"trz{step}")
                    nc.vector.tensor_add(trz[:, :], pgi[:, 0:128],
                                         pgh[:, 0:128])
                    srz = ew.tile([128, 128], F32, tag="srz", name=f"srz{step}")
                    nc.scalar.activation(srz[:, :], trz[:, :], AF.Sigmoid)
                    x1 = ew.tile([128, 64], F32, tag="x1", name=f"x1_{step}")
                    nc.vector.tensor_mul(x1[:, :], srz[:, 0:64],
                                         pgh[:, 128:192])
                    nc.vector.tensor_add(x1[:, :], x1[:, :], pgi[:, 128:192])
                    n = ew.tile([128, 64], F32, tag="n", name=f"n{step}")
                    n_inst = nc.scalar.activation(n[:, :], x1[:, :], AF.Tanh)
                    d = ew.tile([128, 64], F32, tag="d", name=f"d{step}")
                    nc.vector.tensor_sub(d[:, :], hg_prev[:, :], n[:, :])
                    nc.vector.tensor_mul(d[:, :], srz[:, 64:128], d[:, :])
                    hgn = stpool.tile([128, 64], F32, tag="hg_st",
                                      name=f"hg_st{step}")
                    nc.vector.tensor_add(hgn[:, :], n[:, :], d[:, :])
                    hgb = ew.tile([128, 64], BF, tag="hgb", name=f"hgb{step}")
                    nc.vector.tensor_copy(hgb[:, :], hgn[:, :])
                    y_hg = ew.tile([128, 64], BF, tag="y_hg",
                                   name=f"y_hg{step}")
                    nc.vector.transpose(y_hg[:, :], hgb[:, :])
                return hgn, y_hg, n_inst

            def emit_bounce_out(bounce, sec, y, engine):
                # bounce[sec*HS + 128*c2 + r, b] = y[r, 32*c2 + b]
                return engine.dma_start(
                    bounce[sec * HS:(sec + 1) * HS, :].rearrange(
                        "(c r) b -> r c b", c=2),
                    y[:, :].rearrange("r (c b) -> r c b", c=2))

            def emit_bounce_hc(bounce, y, engine):
                # bounce[hc*HS + 128*c2 + r, b] = y[r, hc*64 + 32*c2 + b]
                return engine.dma_start(
                    bounce[:, :].rearrange("(hc c r) b -> r hc c b", hc=2, c=2),
                    y[:, :].rearrange("r (hc c b) -> r hc c b", hc=2, c=2))

            def emit_gather_in(gath, nsec, sec, dst2, eng0, eng1):
                g5 = gath.ap().rearrange(
                    "(rank s half r) b -> s half r rank b", rank=NC, s=nsec,
                    half=2)
                i0 = eng0.dma_start(dst2(0), g5[sec][0])
                i1 = eng1.dma_start(dst2(1), g5[sec][1])
                return i0, i1

            def emit_decode_vt(c, vt, ncols):
                h = hists[c % 2]
                pd = psd.tile([128, 256], F32, tag="ps_d", name=f"ps_d{c}_{vt}")
                for k in range(KT):
                    nc.tensor.matmul(
                        pd[0:VTW, 0:ncols],
                        linw_sb[:, k * VS + vt * VTW:k * VS + (vt + 1) * VTW],
                        h[:, k * NHIST * B:k * NHIST * B + ncols],
                        start=(k == 0), stop=(k == KT - 1),
                    )
                stg = ew.tile([128, 256], F32, tag="stg", name=f"stg{c}_{vt}")
                nc.scalar.activation(stg[0:VTW, 0:ncols], pd[0:VTW, 0:ncols],
                                     AF.Identity, bias=linb_sb[:, vt:vt + 1])
                nc.gpsimd.dma_start(
                    out[vt * VTW:(vt + 1) * VTW,
                        c * NHIST * B:c * NHIST * B + ncols],
                    stg[0:VTW, 0:ncols])

            ag_rg = [list(range(NC))]

            def emit_ag(bounce, gname, nrows, engine=None):
                gath = nc.dram_tensor(gname, [nrows * NC, B], BF,
                                      addr_space="Shared")
                nc.gpsimd.collective_compute(
                    "AllGather", mybir.AluOpType.bypass,
                    replica_groups=ag_rg,
                    ins=[bounce.ap().opt()], outs=[gath.ap().opt()],
                )
                return gath

            # decode schedule: (chunk c, vocab tile vt) emitted at loop step
            # s = 8c + 9 + vt so it never waits on this step's hist gather
            dec_done = set()

            def emit_decode_for_step(s):
                for vt in range(VT):
                    rem = s - NHIST - 1 - vt
                    if rem >= 0 and rem % NHIST == 0:
                        emit_decode_vt(rem // NHIST, vt, NHIST * B)
                        dec_done.add((rem // NHIST, vt))

            # ---- prologue: LSTM(0) from features ----
            y0 = ew.tile([128, 128], BF, tag="y", name="y0")
            c_st = emit_lstm(0, lambda k: hT_init[:, k * B:(k + 1) * B],
                             feat_blk, y0)
            hg_st = feat_blk
            bounce0 = nc.dram_tensor("bounce0", [2 * HS, B], BF)
            emit_bounce_hc(bounce0, y0, nc.sync)
            gaths_hc = [emit_ag(bounce0, "gathc0", 2 * HS)]
            bounceg0 = nc.dram_tensor("bounceg0", [HS, B], BF)
            nc.sync.dma_start(bounceg0[:, :], featsh_in[:, :])
            gaths_hg = [emit_ag(bounceg0, "gathg0", HS)]

            # ---- main loop ----
            # prev-iteration handles for static queue-order pinning
            prev_n = None        # GRU tail n ACT of iter s-1 (scalar)
            prev_hgb = None      # hg-bounce DMA of iter s-1 (sync)
            for s in range(ts):
                ghc = gaths_hc[s]
                # gather h(s) for LSTM(s+1): halves on sync+scalar (HW DGE)
                if s < ts - 1:
                    hT_st = spool.tile([128, KT * B], BF, tag="hT",
                                       name=f"hT{s}")

                    def hT_half(half, _t=hT_st):
                        return _t[:, :].rearrange(
                            "r (k8 k2 b) -> k2 r k8 b", k8=NC, k2=2)[half]
                    h0, h1 = emit_gather_in(ghc, 2, 0, hT_half,
                                            nc.sync, nc.scalar)
                    if prev_hgb is not None:
                        tile.add_dep_helper(h0.ins, prev_hgb.ins, sync=True,
                                            reason="sync q order")
                    if prev_n is not None:
                        tile.add_dep_helper(h1.ins, prev_n.ins, sync=True,
                                            reason="scalar q order")
                # gather c(s) for GRU(s): half0 gpsimd, half1 scalar
                cT_st = spool.tile([128, KT * B], BF, tag="cT", name=f"cT{s}")

                def cT_half(half, _t=cT_st):
                    return _t[:, :].rearrange(
                        "r (k8 k2 b) -> k2 r k8 b", k8=NC, k2=2)[half]
                c0, c1 = emit_gather_in(ghc, 2, 1, cT_half,
                                        nc.gpsimd, nc.scalar)
                if prev_n is not None:
                    tile.add_dep_helper(c1.ins, prev_n.ins, sync=True,
                                        reason="scalar q order")

                if s < ts - 1:
                    # LSTM(s+1) first on PE; its AG issues during GRU(s)
                    y = ew.tile([128, 128], BF, tag="y", name=f"y{s + 1}")
                    c_st = emit_lstm(
                        s + 1, lambda k: hT_st[:, k * B:(k + 1) * B], c_st, y)
                    with tc.high_priority():
                        bounce = nc.dram_tensor(f"bounce{s + 1}",
                                                [2 * HS, B], BF)
                        emit_bounce_hc(bounce, y, nc.sync)
                        gaths_hc.append(
                            emit_ag(bounce, f"bgathc{s + 1}", 2 * HS))

                # gi matmul right after LSTM: only needs AG_hc(s)
                pgi = emit_gru_gi(s, cT_st)

                # gather hg(s-1) hist slot (from AG_hg(s)): sync+scalar HW
                emit_gather_in(gaths_hg[s], 1, 0,
                               lambda half: hist_slot_half(s - 1, half),
                               nc.sync, nc.scalar)

                # decode fills the PE while gh waits on the hist gather
                emit_decode_for_step(s)

                hg_st, y_hg, prev_n = emit_gru_gh(s, pgi, hg_st)
                with tc.high_priority():
                    bg = nc.dram_tensor(f"bounceg{s + 1}", [HS, B], BF)
                    prev_hgb = emit_bounce_out(bg, 0, y_hg, nc.sync)
                    gaths_hg.append(emit_ag(bg, f"bgathg{s + 1}", HS))

            # ---- epilogue: last hg, leftover decode ----
            mlast = ts - 1
            emit_gather_in(gaths_hg[ts], 1, 0,
                           lambda half: hist_slot_half(mlast, half),
                           nc.sync, nc.scalar)
            nfull = ts // NHIST
            nchunk = nfull + (1 if ts > nfull * NHIST else 0)
            for c in range(nchunk):
                ncols = NHIST * B if c < nfull else (ts - nfull * NHIST) * B
                for vt in range(VT):
                    if (c, vt) not in dec_done:
                        emit_decode_vt(c, vt, ncols)

    nc.compile()
    return nc


def _gate_rows(core, ngates):
    """Global weight-row indices for this core's gate shard, in column order
    (jp, kappa, c2, i) with hidden-local l = 128*c2 + 32*jp + i."""
    jp = np.arange(4)[:, None, None, None]
    kappa = np.arange(ngates)[None, :, None, None]
    c2 = np.arange(2)[None, None, :, None]
    i = np.arange(32)[None, None, None, :]
    rows = kappa * H + core * HS + 128 * c2 + 32 * jp + i
    return rows.reshape(-1)


def _prep_inputs(features, captions, emb, lstm_Wih, lstm_bih, lstm_Whh,
                 lstm_bhh, gru_Wih, gru_bih, gru_Whh, gru_bhh, lin_W, lin_b,
                 ts=TS):
    f32 = np.float32
    features = np.asarray(features, f32)
    captions = np.asarray(captions)
    emb = np.asarray(emb, f32)
    lstm_Wih = np.asarray(lstm_Wih, f32); lstm_bih = np.asarray(lstm_bih, f32)
    lstm_Whh = np.asarray(lstm_Whh, f32); lstm_bhh = np.asarray(lstm_bhh, f32)
    gru_Wih = np.asarray(gru_Wih, f32); gru_bih = np.asarray(gru_bih, f32)
    gru_Whh = np.asarray(gru_Whh, f32); gru_bhh = np.asarray(gru_bhh, f32)
    lin_W = np.asarray(lin_W, f32); lin_b = np.asarray(lin_b, f32)

    xs = emb[captions[:, :ts]]                      # [B, ts, E]
    xs_aug = np.ones((E + 1, ts * B), f32)
    xs_aug[:E, :] = xs.transpose(2, 1, 0).reshape(E, ts * B)

    featT = features.T.copy()                       # [H, B]
    ones = np.ones((1, B), f32)

    in_maps = []
    for core in range(NC):
        rl = _gate_rows(core, 4)
        rg = _gate_rows(core, 3)
        wl = lstm_Whh[rl, :].T
        wih = np.concatenate(
            [lstm_Wih[rl, :].T,
             (lstm_bih[rl] + lstm_bhh[rl])[None, :]], axis=0)
        wgi = gru_Wih[rg, :].T
        brgi = gru_bih[rg].reshape(1, GG)
        wgh = gru_Whh[rg, :].T
        brgh = gru_bhh[rg].reshape(1, GG)
        linw = lin_W[core * VS:(core + 1) * VS, :].T
        linb = lin_b[core * VS:(core + 1) * VS].reshape(VT, VTW).T.copy()
        feat_sh = features[:, core * HS:(core + 1) * HS].T.copy()
        # feat_blk [32*jp+b, 32*c2+i] = features[b, core*HS + 128*c2+32*jp+i]
        fb = features[:, core * HS:(core + 1) * HS].reshape(B, 2, 4, 32)
        feat_blk = fb.transpose(2, 0, 1, 3).reshape(128, 64).copy()

        bf = BF16
        in_maps.append({
            "wl": wl.astype(bf), "wih": wih.astype(bf),
            "wgi": wgi.astype(bf), "brgi": brgi.astype(bf),
            "wgh": wgh.astype(bf), "brgh": brgh.astype(bf),
            "linw": linw.astype(bf),
            "linb": linb.astype(f32),
            "xs_aug": xs_aug.astype(bf),
            "featT": featT.astype(bf),
            "feat_sh": feat_sh.astype(bf),
            "feat_blk": feat_blk.astype(f32),
            "ones": ones.astype(bf),
        })
    return in_maps


def kernel(**inputs):
    ts = TS
    if ts not in _BUILD_CACHE:
        _BUILD_CACHE[ts] = _build(ts)
    nc = _BUILD_CACHE[ts]
    in_maps = _prep_inputs(**inputs, ts=ts)
    res = bass_utils.run_bass_kernel_spmd(nc, in_maps,
                                          core_ids=list(range(NC)))
    full = np.empty((B, ts, V), np.float32)
    for core in range(NC):
        o = res.results[core]["out"]                 # [VS, ts*B]
        full[:, :, core * VS:(core + 1) * VS] = (
            o.reshape(VS, ts, B).transpose(2, 1, 0))
    return full



# revision 12
# speedup vs baseline: 1.0917x; 1.0917x over previous
"""Trainium2 Bass kernel for nn_Decoder (LSTMCell -> GRUCell -> Linear decode).

Strategy (8 NeuronCores, one chip):
  - Hidden dim H=2048 sharded 8 ways (256/core). Each core holds the weight
    rows for its hidden slice of the LSTM/GRU gates in SBUF (bf16), computes
    its gate shard with batch-major col-tiled matmuls (stationary = h-major
    state tiles [128,32], moving = weight columns), applies the elementwise
    cell updates in fp32, stream-transposes its new state shard to h-major
    bf16 and exchanges it via two pipelined AllGathers per step: AG_hc
    carries [h(s+1), c(s+1)] (issued right after the LSTM, while the GRU
    still computes) and AG_hg carries [hg(s)].
  - Hidden-index layout l = 128*c2 + 32*jp + i is chosen so the 32x32-block
    StreamTranspose of the [128,64] state tile directly yields the h-major
    shard, and every DRAM exchange is a single strided DMA.
  - The vocab-sharded linear decode (1000 rows/core) consumes a double-
    buffered history of gathered hg, one vocab tile per step, hiding under
    the collective latency and keeping TensorE warm.
  - kernel(**inputs) takes FULL inputs, shards on host, runs the SPMD NEFF
    on cores 0-7 via run_bass_kernel_spmd, reassembles the FULL output.
"""
import os
import sys

import numpy as np

for _p in ("/root/.axon_site", "/root/.axon_site/_ro/trn_rl_repo",
           "/root/.axon_site/_ro/pypackages", "/opt/trn_rl_repo"):
    if os.path.isdir(_p) and _p not in sys.path:
        sys.path.append(_p)

import concourse.bacc as bacc
import concourse.bass as bass
import concourse.mybir as mybir
import concourse.tile as tile
from concourse import bass_utils

import ml_dtypes

BF16 = ml_dtypes.bfloat16
F32 = mybir.dt.float32
BF = mybir.dt.bfloat16
AF = mybir.ActivationFunctionType

NC = 8          # cores
B = 32          # batch
T = 40          # caption length
TS = T - 1      # recurrent steps
V = 8000
E = 50
H = 2048
HS = H // NC    # 256 hidden per core
VS = V // NC    # 1000 vocab per core
KT = H // 128   # 16 contraction tiles
VT = 8          # vocab tiles per core
VTW = VS // VT  # 125 cols per vocab tile
GL = 4 * HS     # 1024 lstm gate cols per core
GG = 3 * HS     # 768 gru gate cols per matmul per core
NHIST = 8       # decode chunk length

_BUILD_CACHE = {}


def _build(ts=TS):
    nc = bacc.Bacc("TRN2", target_bir_lowering=False, debug=False,
                   enable_asserts=True, num_devices=NC)

    # ---- external I/O (per core) ----
    wl_in = nc.dram_tensor("wl", [H, GL], BF, kind="ExternalInput")
    wih_in = nc.dram_tensor("wih", [E + 1, GL], BF, kind="ExternalInput")
    wgi_in = nc.dram_tensor("wgi", [H, GG], BF, kind="ExternalInput")
    brgi_in = nc.dram_tensor("brgi", [1, GG], BF, kind="ExternalInput")
    wgh_in = nc.dram_tensor("wgh", [H, GG], BF, kind="ExternalInput")
    brgh_in = nc.dram_tensor("brgh", [1, GG], BF, kind="ExternalInput")
    linw_in = nc.dram_tensor("linw", [H, VS], BF, kind="ExternalInput")
    linb_in = nc.dram_tensor("linb", [VTW, VT], F32, kind="ExternalInput")
    xs_in = nc.dram_tensor("xs_aug", [E + 1, ts * B], BF, kind="ExternalInput")
    featT_in = nc.dram_tensor("featT", [H, B], BF, kind="ExternalInput")
    featsh_in = nc.dram_tensor("feat_sh", [HS, B], BF, kind="ExternalInput")
    featblk_in = nc.dram_tensor("feat_blk", [128, 64], F32, kind="ExternalInput")
    ones_in = nc.dram_tensor("ones", [1, B], BF, kind="ExternalInput")
    out = nc.dram_tensor("out", [VS, ts * B], F32, kind="ExternalOutput")

    with tile.TileContext(nc) as tc:
        with (
            tc.tile_pool(name="const", bufs=1) as cpool,
            tc.tile_pool(name="stat", bufs=3) as spool,
            tc.tile_pool(name="state", bufs=2) as stpool,
            tc.tile_pool(name="ew", bufs=3) as ew,
            tc.tile_pool(name="psl", bufs=2, space="PSUM") as psl,
            tc.tile_pool(name="psg", bufs=2, space="PSUM") as psg,
            tc.tile_pool(name="psd", bufs=2, space="PSUM") as psd,
        ):
            # ---- load weights / constants into SBUF (single strided DMAs) --
            wl_sb = cpool.tile([128, KT * GL], BF)
            nc.sync.dma_start(
                wl_sb[:, :].rearrange("r (k c) -> r k c", k=KT),
                wl_in[:, :].rearrange("(k r) c -> r k c", k=KT))
            wgi_sb = cpool.tile([128, KT * GG], BF)
            nc.sync.dma_start(
                wgi_sb[:, :].rearrange("r (k c) -> r k c", k=KT),
                wgi_in[:, :].rearrange("(k r) c -> r k c", k=KT))
            wgh_sb = cpool.tile([128, KT * GG], BF)
            nc.sync.dma_start(
                wgh_sb[:, :].rearrange("r (k c) -> r k c", k=KT),
                wgh_in[:, :].rearrange("(k r) c -> r k c", k=KT))
            linw_sb = cpool.tile([128, KT * VS], BF)
            nc.scalar.dma_start(
                linw_sb[:, :].rearrange("r (k c) -> r k c", k=KT),
                linw_in[:, :].rearrange("(k r) c -> r k c", k=KT))
            wih_sb = cpool.tile([E + 1, GL], BF)
            nc.sync.dma_start(wih_sb[:, :], wih_in[:, :])
            brgi_sb = cpool.tile([1, GG], BF)
            nc.sync.dma_start(brgi_sb[:, :], brgi_in[:, :])
            brgh_sb = cpool.tile([1, GG], BF)
            nc.sync.dma_start(brgh_sb[:, :], brgh_in[:, :])
            linb_sb = cpool.tile([VTW, VT], F32)
            nc.scalar.dma_start(linb_sb[:, :], linb_in[:, :])
            xs_sb = cpool.tile([E + 1, ts * B], BF)
            nc.sync.dma_start(xs_sb[:, :], xs_in[:, :])
            ones_sb = cpool.tile([1, B], BF)
            nc.sync.dma_start(ones_sb[:, :], ones_in[:, :])
            feat_blk = cpool.tile([128, 64], F32)
            nc.sync.dma_start(feat_blk[:, :], featblk_in[:, :])
            hT_init = cpool.tile([128, KT * B], BF)
            nc.sync.dma_start(
                hT_init[:, :].rearrange("r (k b) -> r k b", k=KT),
                featT_in[:, :].rearrange("(k r) b -> r k b", k=KT))
            # double-buffered gathered-hg history (h-major, bf16)
            hists = [cpool.tile([128, KT * NHIST * B], BF, name=f"hist{p}")
                     for p in range(2)]

            def hist_slot_half(m, half):
                """DMA-dst AP (r, k8, b) for hg(m)'s history slot, k%2==half."""
                h = hists[(m // NHIST) % 2]
                return h[:, :].rearrange(
                    "r (k8 k2 t b) -> k2 t r k8 b", k8=NC, k2=2,
                    t=NHIST)[half][m % NHIST]

            def hist_slot_k(m, k):
                """Stationary AP [128, B] for hg(m) k-tile k."""
                h = hists[(m // NHIST) % 2]
                return h[:, k * NHIST * B + (m % NHIST) * B:
                         k * NHIST * B + (m % NHIST) * B + B]

            def emit_lstm(step, hT_ap, c_prev, y):
                gsum = psl.tile([128, 256], F32, tag="ps_l", name=f"ps_l{step}")
                for k in range(KT):
                    for j in range(4):
                        nc.tensor.matmul(
                            gsum[32 * j:32 * j + 32, :],
                            hT_ap(k),
                            wl_sb[:, k * GL + j * 256:k * GL + j * 256 + 256],
                            start=(k == 0), stop=False,
                            tile_position=(0, 32 * j),
                        )
                for j in range(4):
                    nc.tensor.matmul(
                        gsum[32 * j:32 * j + 32, :],
                        xs_sb[:, step * B:(step + 1) * B],
                        wih_sb[:, j * 256:j * 256 + 256],
                        start=False, stop=True,
                        tile_position=(0, 32 * j),
                    )
                tc2 = tc
                with tc2.high_priority():
                    return emit_lstm_tail(step, gsum, c_prev, y)

            def emit_lstm_tail(step, gsum, c_prev, y):
                tg = ew.tile([128, 64], F32, tag="tg", name=f"tg{step}")
                nc.scalar.activation(tg[:, :], gsum[:, 128:192], AF.Tanh)
                sif = ew.tile([128, 128], F32, tag="sif", name=f"sif{step}")
                nc.scalar.activation(sif[:, :], gsum[:, 0:128], AF.Sigmoid)
                so = ew.tile([128, 64], F32, tag="so", name=f"so{step}")
                nc.scalar.activation(so[:, :], gsum[:, 192:256], AF.Sigmoid)
                t1 = ew.tile([128, 64], F32, tag="t1", name=f"t1_{step}")
                nc.vector.tensor_mul(t1[:, :], sif[:, 0:64], tg[:, :])
                cn = stpool.tile([128, 64], F32, tag="c_st", name=f"c_st{step}")
                nc.vector.tensor_mul(cn[:, :], sif[:, 64:128], c_prev[:, :])
                nc.vector.tensor_add(cn[:, :], cn[:, :], t1[:, :])
                cb = ew.tile([128, 64], BF, tag="cb", name=f"cb{step}")
                nc.vector.tensor_copy(cb[:, :], cn[:, :])
                nc.vector.transpose(y[:, 64:128], cb[:, :])
                tc_ = ew.tile([128, 64], F32, tag="tc", name=f"tc{step}")
                nc.scalar.activation(tc_[:, :], cn[:, :], AF.Tanh)
                hb = ew.tile([128, 64], BF, tag="hb", name=f"hb{step}")
                nc.vector.tensor_mul(hb[:, :], so[:, :], tc_[:, :])
                nc.vector.transpose(y[:, 0:64], hb[:, :])
                return cn

            def emit_gru_gi(step, cT_st):
                """gi matmul (+bias via ones-row) — depends only on AG_hc."""
                pgi = psg.tile([128, 192], F32, tag="ps_gi", name=f"ps_gi{step}")
                for k in range(KT):
                    for j in range(4):
                        nc.tensor.matmul(
                            pgi[32 * j:32 * j + 32, :],
                            cT_st[:, k * B:(k + 1) * B],
                            wgi_sb[:, k * GG + j * 192:k * GG + j * 192 + 192],
                            start=(k == 0), stop=False,
                            tile_position=(0, 32 * j),
                        )
                for j in range(4):
                    nc.tensor.matmul(
                        pgi[32 * j:32 * j + 32, :],
                        ones_sb[:, :],
                        brgi_sb[:, j * 192:(j + 1) * 192],
                        start=False, stop=(j == 3),
                        tile_position=(0, 32 * j),
                    )
                gi_sb = ew.tile([128, 192], F32, tag="gi_sb",
                                name=f"gi_sb{step}")
                nc.vector.tensor_copy(gi_sb[:, :], pgi[:, :])
                return gi_sb

            def emit_gru_gh(step, pgi, hg_prev):
                """gh matmul + elementwise tail (the latency-critical part)."""
                pgh = psg.tile([128, 192], F32, tag="ps_gh", name=f"ps_gh{step}")
                for k in range(KT):
                    for j in range(4):
                        nc.tensor.matmul(
                            pgh[32 * j:32 * j + 32, :],
                            hist_slot_k(step - 1, k),
                            wgh_sb[:, k * GG + j * 192:k * GG + j * 192 + 192],
                            start=(k == 0), stop=False,
                            tile_position=(0, 32 * j),
                        )
                for j in range(4):
                    nc.tensor.matmul(
                        pgh[32 * j:32 * j + 32, :],
                        ones_sb[:, :],
                        brgh_sb[:, j * 192:(j + 1) * 192],
                        start=False, stop=(j == 3),
                        tile_position=(0, 32 * j),
                    )
                with tc.high_priority():
                    trz = ew.tile([128, 128], F32, tag="trz", name=f"trz{step}")
                    nc.vector.tensor_add(trz[:, :], pgi[:, 0:128],
                                         pgh[:, 0:128])
                    srz = ew.tile([128, 128], F32, tag="srz", name=f"srz{step}")
                    nc.scalar.activation(srz[:, :], trz[:, :], AF.Sigmoid)
                    x1 = ew.tile([128, 64], F32, tag="x1", name=f"x1_{step}")
                    nc.vector.tensor_mul(x1[:, :], srz[:, 0:64],
                                         pgh[:, 128:192])
                    nc.vector.tensor_add(x1[:, :], x1[:, :], pgi[:, 128:192])
                    n = ew.tile([128, 64], F32, tag="n", name=f"n{step}")
                    n_inst = nc.scalar.activation(n[:, :], x1[:, :], AF.Tanh)
                    d = ew.tile([128, 64], F32, tag="d", name=f"d{step}")
                    nc.vector.tensor_sub(d[:, :], hg_prev[:, :], n[:, :])
                    nc.vector.tensor_mul(d[:, :], srz[:, 64:128], d[:, :])
                    hgn = stpool.tile([128, 64], F32, tag="hg_st",
                                      name=f"hg_st{step}")
                    nc.vector.tensor_add(hgn[:, :], n[:, :], d[:, :])
                    hgb = ew.tile([128, 64], BF, tag="hgb", name=f"hgb{step}")
                    nc.vector.tensor_copy(hgb[:, :], hgn[:, :])
                    y_hg = ew.tile([128, 64], BF, tag="y_hg",
                                   name=f"y_hg{step}")
                    nc.vector.transpose(y_hg[:, :], hgb[:, :])
                return hgn, y_hg, n_inst

            def emit_bounce_out(bounce, sec, y, engine):
                # bounce[sec*HS + 128*c2 + r, b] = y[r, 32*c2 + b]
                return engine.dma_start(
                    bounce[sec * HS:(sec + 1) * HS, :].rearrange(
                        "(c r) b -> r c b", c=2),
                    y[:, :].rearrange("r (c b) -> r c b", c=2))

            def emit_bounce_hc(bounce, y, engine):
                # bounce[hc*HS + 128*c2 + r, b] = y[r, hc*64 + 32*c2 + b]
                return engine.dma_start(
                    bounce[:, :].rearrange("(hc c r) b -> r hc c b", hc=2, c=2),
                    y[:, :].rearrange("r (hc c b) -> r hc c b", hc=2, c=2))

            def emit_gather_in(gath, nsec, sec, dst2, eng0, eng1):
                g5 = gath.ap().rearrange(
                    "(rank s half r) b -> s half r rank b", rank=NC, s=nsec,
                    half=2)
                i0 = eng0.dma_start(dst2(0), g5[sec][0])
                i1 = eng1.dma_start(dst2(1), g5[sec][1])
                return i0, i1

            pend_dec = {}

            def emit_decode_mm(c, vt, ncols):
                h = hists[c % 2]
                pd = psd.tile([128, 256], F32, tag="ps_d", name=f"ps_d{c}_{vt}")
                for k in range(KT):
                    nc.tensor.matmul(
                        pd[0:VTW, 0:ncols],
                        linw_sb[:, k * VS + vt * VTW:k * VS + (vt + 1) * VTW],
                        h[:, k * NHIST * B:k * NHIST * B + ncols],
                        start=(k == 0), stop=(k == KT - 1),
                    )
                pend_dec[(c, vt)] = (pd, ncols)

            def emit_decode_fin(c, vt):
                pd, ncols = pend_dec.pop((c, vt))
                stg = ew.tile([128, 256], F32, tag="stg", name=f"stg{c}_{vt}")
                nc.scalar.activation(stg[0:VTW, 0:ncols], pd[0:VTW, 0:ncols],
                                     AF.Identity, bias=linb_sb[:, vt:vt + 1])
                nc.gpsimd.dma_start(
                    out[vt * VTW:(vt + 1) * VTW,
                        c * NHIST * B:c * NHIST * B + ncols],
                    stg[0:VTW, 0:ncols])

            def emit_decode_vt(c, vt, ncols):
                emit_decode_mm(c, vt, ncols)
                emit_decode_fin(c, vt)

            ag_rg = [list(range(NC))]

            def emit_ag(bounce, gname, nrows, engine=None):
                gath = nc.dram_tensor(gname, [nrows * NC, B], BF,
                                      addr_space="Shared")
                nc.gpsimd.collective_compute(
                    "AllGather", mybir.AluOpType.bypass,
                    replica_groups=ag_rg,
                    ins=[bounce.ap().opt()], outs=[gath.ap().opt()],
                )
                return gath

            # decode schedule: (chunk c, vocab tile vt) emitted at loop step
            # s = 8c + 9 + vt so it never waits on this step's hist gather
            dec_done = set()

            def emit_decode_for_step(s):
                for vt in range(VT):
                    rem = s - NHIST - 1 - vt
                    if rem >= 0 and rem % NHIST == 0:
                        emit_decode_mm(rem // NHIST, vt, NHIST * B)
                        dec_done.add((rem // NHIST, vt))

            def emit_decode_fin_for_step(s):
                for vt in range(VT):
                    rem = s - NHIST - 1 - vt
                    if rem >= 0 and rem % NHIST == 0:
                        emit_decode_fin(rem // NHIST, vt)

            # ---- prologue: LSTM(0) from features ----
            y0 = ew.tile([128, 128], BF, tag="y", name="y0")
            c_st = emit_lstm(0, lambda k: hT_init[:, k * B:(k + 1) * B],
                             feat_blk, y0)
            hg_st = feat_blk
            bounce0 = nc.dram_tensor("bounce0", [2 * HS, B], BF)
            emit_bounce_hc(bounce0, y0, nc.sync)
            gaths_hc = [emit_ag(bounce0, "gathc0", 2 * HS)]
            bounceg0 = nc.dram_tensor("bounceg0", [HS, B], BF)
            nc.sync.dma_start(bounceg0[:, :], featsh_in[:, :])
            gaths_hg = [emit_ag(bounceg0, "gathg0", HS)]

            # ---- main loop ----
            # prev-iteration handles for static queue-order pinning
            prev_n = None        # GRU tail n ACT of iter s-1 (scalar)
            prev_hgb = None      # hg-bounce DMA of iter s-1 (sync)
            for s in range(ts):
                ghc = gaths_hc[s]
                # gather h(s) for LSTM(s+1): halves on sync+scalar (HW DGE)
                if s < ts - 1:
                    hT_st = spool.tile([128, KT * B], BF, tag="hT",
                                       name=f"hT{s}")

                    def hT_half(half, _t=hT_st):
                        return _t[:, :].rearrange(
                            "r (k8 k2 b) -> k2 r k8 b", k8=NC, k2=2)[half]
                    h0, h1 = emit_gather_in(ghc, 2, 0, hT_half,
                                            nc.sync, nc.scalar)
                    if prev_n is not None:
                        tile.add_dep_helper(h1.ins, prev_n.ins, sync=True,
                                            reason="scalar q order")
                # gather c(s) for GRU(s): half0 gpsimd, half1 scalar
                cT_st = spool.tile([128, KT * B], BF, tag="cT", name=f"cT{s}")

                def cT_half(half, _t=cT_st):
                    return _t[:, :].rearrange(
                        "r (k8 k2 b) -> k2 r k8 b", k8=NC, k2=2)[half]
                c0, c1 = emit_gather_in(ghc, 2, 1, cT_half,
                                        nc.gpsimd, nc.scalar)
                if prev_n is not None:
                    tile.add_dep_helper(c1.ins, prev_n.ins, sync=True,
                                        reason="scalar q order")

                if s < ts - 1:
                    # LSTM(s+1) first on PE; its AG issues during GRU(s)
                    y = ew.tile([128, 128], BF, tag="y", name=f"y{s + 1}")
                    c_st = emit_lstm(
                        s + 1, lambda k: hT_st[:, k * B:(k + 1) * B], c_st, y)
                    with tc.high_priority():
                        bounce = nc.dram_tensor(f"bounce{s + 1}",
                                                [2 * HS, B], BF)
                        emit_bounce_hc(bounce, y, nc.sync)
                        gaths_hc.append(
                            emit_ag(bounce, f"bgathc{s + 1}", 2 * HS))

                # gi matmul right after LSTM: only needs AG_hc(s)
                pgi = emit_gru_gi(s, cT_st)

                # gather hg(s-1) hist slot (from AG_hg(s)): sync+scalar HW
                emit_gather_in(gaths_hg[s], 1, 0,
                               lambda half: hist_slot_half(s - 1, half),
                               nc.gpsimd, nc.sync)

                # decode fills the PE while gh waits on the hist gather
                emit_decode_for_step(s)

                hg_st, y_hg, prev_n = emit_gru_gh(s, pgi, hg_st)
                with tc.high_priority():
                    bg = nc.dram_tensor(f"bounceg{s + 1}", [HS, B], BF)
                    prev_hgb = emit_bounce_out(bg, 0, y_hg, nc.scalar)
                    gaths_hg.append(emit_ag(bg, f"bgathg{s + 1}", HS))
                emit_decode_fin_for_step(s)

            # ---- epilogue: last hg, leftover decode ----
            mlast = ts - 1
            emit_gather_in(gaths_hg[ts], 1, 0,
                           lambda half: hist_slot_half(mlast, half),
                           nc.gpsimd, nc.sync)
            nfull = ts // NHIST
            nchunk = nfull + (1 if ts > nfull * NHIST else 0)
            for c in range(nchunk):
                ncols = NHIST * B if c < nfull else (ts - nfull * NHIST) * B
                for vt in range(VT):
                    if (c, vt) not in dec_done:
                        emit_decode_vt(c, vt, ncols)

    nc.compile()
    return nc


def _gate_rows(core, ngates):
    """Global weight-row indices for this core's gate shard, in column order
    (jp, kappa, c2, i) with hidden-local l = 128*c2 + 32*jp + i."""
    jp = np.arange(4)[:, None, None, None]
    kappa = np.arange(ngates)[None, :, None, None]
    c2 = np.arange(2)[None, None, :, None]
    i = np.arange(32)[None, None, None, :]
    rows = kappa * H + core * HS + 128 * c2 + 32 * jp + i
    return rows.reshape(-1)


def _prep_inputs(features, captions, emb, lstm_Wih, lstm_bih, lstm_Whh,
                 lstm_bhh, gru_Wih, gru_bih, gru_Whh, gru_bhh, lin_W, lin_b,
                 ts=TS):
    f32 = np.float32
    features = np.asarray(features, f32)
    captions = np.asarray(captions)
    emb = np.asarray(emb, f32)
    lstm_Wih = np.asarray(lstm_Wih, f32); lstm_bih = np.asarray(lstm_bih, f32)
    lstm_Whh = np.asarray(lstm_Whh, f32); lstm_bhh = np.asarray(lstm_bhh, f32)
    gru_Wih = np.asarray(gru_Wih, f32); gru_bih = np.asarray(gru_bih, f32)
    gru_Whh = np.asarray(gru_Whh, f32); gru_bhh = np.asarray(gru_bhh, f32)
    lin_W = np.asarray(lin_W, f32); lin_b = np.asarray(lin_b, f32)

    xs = emb[captions[:, :ts]]                      # [B, ts, E]
    xs_aug = np.ones((E + 1, ts * B), f32)
    xs_aug[:E, :] = xs.transpose(2, 1, 0).reshape(E, ts * B)

    featT = features.T.copy()                       # [H, B]
    ones = np.ones((1, B), f32)

    in_maps = []
    for core in range(NC):
        rl = _gate_rows(core, 4)
        rg = _gate_rows(core, 3)
        wl = lstm_Whh[rl, :].T
        wih = np.concatenate(
            [lstm_Wih[rl, :].T,
             (lstm_bih[rl] + lstm_bhh[rl])[None, :]], axis=0)
        wgi = gru_Wih[rg, :].T
        brgi = gru_bih[rg].reshape(1, GG)
        wgh = gru_Whh[rg, :].T
        brgh = gru_bhh[rg].reshape(1, GG)
        linw = lin_W[core * VS:(core + 1) * VS, :].T
        linb = lin_b[core * VS:(core + 1) * VS].reshape(VT, VTW).T.copy()
        feat_sh = features[:, core * HS:(core + 1) * HS].T.copy()
        # feat_blk [32*jp+b, 32*c2+i] = features[b, core*HS + 128*c2+32*jp+i]
        fb = features[:, core * HS:(core + 1) * HS].reshape(B, 2, 4, 32)
        feat_blk = fb.transpose(2, 0, 1, 3).reshape(128, 64).copy()

        bf = BF16
        in_maps.append({
            "wl": wl.astype(bf), "wih": wih.astype(bf),
            "wgi": wgi.astype(bf), "brgi": brgi.astype(bf),
            "wgh": wgh.astype(bf), "brgh": brgh.astype(bf),
            "linw": linw.astype(bf),
            "linb": linb.astype(f32),
            "xs_aug": xs_aug.astype(bf),
            "featT": featT.astype(bf),
            "feat_sh": feat_sh.astype(bf),
            "feat_blk": feat_blk.astype(f32),
            "ones": ones.astype(bf),
        })
    return in_maps


def kernel(**inputs):
    ts = TS
    if ts not in _BUILD_CACHE:
        _BUILD_CACHE[ts] = _build(ts)
    nc = _BUILD_CACHE[ts]
    in_maps = _prep_inputs(**inputs, ts=ts)
    res = bass_utils.run_bass_kernel_spmd(nc, in_maps,
                                          core_ids=list(range(NC)))
    full = np.empty((B, ts, V), np.float32)
    for core in range(NC):
        o = res.results[core]["out"]                 # [VS, ts*B]
        full[:, :, core * VS:(core + 1) * VS] = (
            o.reshape(VS, ts, B).transpose(2, 1, 0))
    return full



# revision 14
# speedup vs baseline: 1.1325x; 1.0373x over previous
"""Trainium2 Bass kernel for nn_Decoder (LSTMCell -> GRUCell -> Linear decode).

Strategy (8 NeuronCores, one chip):
  - Hidden dim H=2048 sharded 8 ways (256/core). Each core holds the weight
    rows for its hidden slice of the LSTM/GRU gates in SBUF (bf16), computes
    its gate shard with batch-major col-tiled matmuls (stationary = h-major
    state tiles [128,32], moving = weight columns), applies the elementwise
    cell updates in fp32, stream-transposes its new state shard to h-major
    bf16 and exchanges it via two pipelined AllGathers per step: AG_hc
    carries [h(s+1), c(s+1)] (issued right after the LSTM, while the GRU
    still computes) and AG_hg carries [hg(s)].
  - Hidden-index layout l = 128*c2 + 32*jp + i is chosen so the 32x32-block
    StreamTranspose of the [128,64] state tile directly yields the h-major
    shard, and every DRAM exchange is a single strided DMA.
  - The vocab-sharded linear decode (1000 rows/core) consumes a double-
    buffered history of gathered hg, one vocab tile per step, hiding under
    the collective latency and keeping TensorE warm.
  - kernel(**inputs) takes FULL inputs, shards on host, runs the SPMD NEFF
    on cores 0-7 via run_bass_kernel_spmd, reassembles the FULL output.
"""
import os
import sys

import numpy as np

for _p in ("/root/.axon_site", "/root/.axon_site/_ro/trn_rl_repo",
           "/root/.axon_site/_ro/pypackages", "/opt/trn_rl_repo"):
    if os.path.isdir(_p) and _p not in sys.path:
        sys.path.append(_p)

import concourse.bacc as bacc
import concourse.bass as bass
import concourse.mybir as mybir
import concourse.tile as tile
from concourse import bass_utils

import ml_dtypes

BF16 = ml_dtypes.bfloat16
F32 = mybir.dt.float32
BF = mybir.dt.bfloat16
AF = mybir.ActivationFunctionType

NC = 8          # cores
B = 32          # batch
T = 40          # caption length
TS = T - 1      # recurrent steps
V = 8000
E = 50
H = 2048
HS = H // NC    # 256 hidden per core
VS = V // NC    # 1000 vocab per core
KT = H // 128   # 16 contraction tiles
VT = 8          # vocab tiles per core
VTW = VS // VT  # 125 cols per vocab tile
GL = 4 * HS     # 1024 lstm gate cols per core
GG = 3 * HS     # 768 gru gate cols per matmul per core
NHIST = 8       # decode chunk length

_BUILD_CACHE = {}


def _build(ts=TS):
    nc = bacc.Bacc("TRN2", target_bir_lowering=False, debug=False,
                   enable_asserts=True, num_devices=NC)

    # ---- external I/O (per core) ----
    wl_in = nc.dram_tensor("wl", [H, GL], BF, kind="ExternalInput")
    wih_in = nc.dram_tensor("wih", [E + 1, GL], BF, kind="ExternalInput")
    wgi_in = nc.dram_tensor("wgi", [H, GG], BF, kind="ExternalInput")
    brgi_in = nc.dram_tensor("brgi", [1, GG], BF, kind="ExternalInput")
    wgh_in = nc.dram_tensor("wgh", [H, GG], BF, kind="ExternalInput")
    brgh_in = nc.dram_tensor("brgh", [1, GG], BF, kind="ExternalInput")
    linw_in = nc.dram_tensor("linw", [H, VS], BF, kind="ExternalInput")
    linb_in = nc.dram_tensor("linb", [VTW, VT], F32, kind="ExternalInput")
    xs_in = nc.dram_tensor("xs_aug", [E + 1, ts * B], BF, kind="ExternalInput")
    featT_in = nc.dram_tensor("featT", [H, B], BF, kind="ExternalInput")
    featsh_in = nc.dram_tensor("feat_sh", [128, 64], BF, kind="ExternalInput")
    featblk_in = nc.dram_tensor("feat_blk", [128, 64], F32, kind="ExternalInput")
    ones_in = nc.dram_tensor("ones", [1, B], BF, kind="ExternalInput")
    out = nc.dram_tensor("out", [VS, ts * B], F32, kind="ExternalOutput")

    with tile.TileContext(nc) as tc:
        with (
            tc.tile_pool(name="const", bufs=1) as cpool,
            tc.tile_pool(name="stat", bufs=3) as spool,
            tc.tile_pool(name="state", bufs=2) as stpool,
            tc.tile_pool(name="ew", bufs=3) as ew,
            tc.tile_pool(name="psl", bufs=2, space="PSUM") as psl,
            tc.tile_pool(name="psg", bufs=2, space="PSUM") as psg,
            tc.tile_pool(name="psd", bufs=2, space="PSUM") as psd,
        ):
            # warmup collective: absorbs the ~12us cold-start of the CC
            # stream while the weight DMAs stream in
            ones_sb = cpool.tile([1, B], BF)
            nc.sync.dma_start(ones_sb[:, :], ones_in[:, :])
            warm_in = nc.dram_tensor("warmi", [1, B], BF)
            nc.sync.dma_start(warm_in[:, :], ones_sb[:, :])
            warm_out = nc.dram_tensor("warmg", [NC, B], BF,
                                      addr_space="Shared")
            nc.gpsimd.collective_compute(
                "AllGather", mybir.AluOpType.bypass,
                replica_groups=[list(range(NC))],
                ins=[warm_in.ap().opt()], outs=[warm_out.ap().opt()])

            # ---- load weights / constants into SBUF (single strided DMAs) --
            wl_sb = cpool.tile([128, KT * GL], BF)
            nc.sync.dma_start(
                wl_sb[:, :].rearrange("r (k c) -> r k c", k=KT),
                wl_in[:, :].rearrange("(k r) c -> r k c", k=KT))
            wgi_sb = cpool.tile([128, KT * GG], BF)
            nc.sync.dma_start(
                wgi_sb[:, :].rearrange("r (k c) -> r k c", k=KT),
                wgi_in[:, :].rearrange("(k r) c -> r k c", k=KT))
            wgh_sb = cpool.tile([128, KT * GG], BF)
            nc.sync.dma_start(
                wgh_sb[:, :].rearrange("r (k c) -> r k c", k=KT),
                wgh_in[:, :].rearrange("(k r) c -> r k c", k=KT))
            linw_sb = cpool.tile([128, KT * VS], BF)
            nc.scalar.dma_start(
                linw_sb[:, :].rearrange("r (k c) -> r k c", k=KT),
                linw_in[:, :].rearrange("(k r) c -> r k c", k=KT))
            wih_sb = cpool.tile([E + 1, GL], BF)
            nc.sync.dma_start(wih_sb[:, :], wih_in[:, :])
            brgi_sb = cpool.tile([1, GG], BF)
            nc.sync.dma_start(brgi_sb[:, :], brgi_in[:, :])
            brgh_sb = cpool.tile([1, GG], BF)
            nc.sync.dma_start(brgh_sb[:, :], brgh_in[:, :])
            linb_sb = cpool.tile([VTW, VT], F32)
            nc.scalar.dma_start(linb_sb[:, :], linb_in[:, :])
            xs_sb = cpool.tile([E + 1, ts * B], BF)
            nc.sync.dma_start(xs_sb[:, :], xs_in[:, :])
            feat_blk = cpool.tile([128, 64], F32)
            nc.sync.dma_start(feat_blk[:, :], featblk_in[:, :])
            hT_init = cpool.tile([128, KT * B], BF)
            nc.sync.dma_start(
                hT_init[:, :].rearrange("r (k b) -> r k b", k=KT),
                featT_in[:, :].rearrange("(k r) b -> r k b", k=KT))
            # double-buffered gathered-hg history (h-major, bf16)
            hists = [cpool.tile([128, KT * NHIST * B], BF, name=f"hist{p}")
                     for p in range(2)]

            def hist_slot_half(m, half):
                """DMA-dst AP (r, k8, b) for hg(m)'s history slot, k%2==half."""
                h = hists[(m // NHIST) % 2]
                return h[:, :].rearrange(
                    "r (k8 k2 t b) -> k2 t r k8 b", k8=NC, k2=2,
                    t=NHIST)[half][m % NHIST]

            def hist_slot_k(m, k):
                """Stationary AP [128, B] for hg(m) k-tile k."""
                h = hists[(m // NHIST) % 2]
                return h[:, k * NHIST * B + (m % NHIST) * B:
                         k * NHIST * B + (m % NHIST) * B + B]

            def emit_lstm(step, hT_ap, c_prev, y):
                gsum = psl.tile([128, 256], F32, tag="ps_l", name=f"ps_l{step}")
                for k in range(KT):
                    for j in range(4):
                        nc.tensor.matmul(
                            gsum[32 * j:32 * j + 32, :],
                            hT_ap(k),
                            wl_sb[:, k * GL + j * 256:k * GL + j * 256 + 256],
                            start=(k == 0), stop=False,
                            tile_position=(0, 32 * j),
                        )
                for j in range(4):
                    nc.tensor.matmul(
                        gsum[32 * j:32 * j + 32, :],
                        xs_sb[:, step * B:(step + 1) * B],
                        wih_sb[:, j * 256:j * 256 + 256],
                        start=False, stop=True,
                        tile_position=(0, 32 * j),
                    )
                tc2 = tc
                with tc2.high_priority():
                    return emit_lstm_tail(step, gsum, c_prev, y)

            def emit_lstm_tail(step, gsum, c_prev, y):
                tg = ew.tile([128, 64], F32, tag="tg", name=f"tg{step}")
                nc.scalar.activation(tg[:, :], gsum[:, 128:192], AF.Tanh)
                sif = ew.tile([128, 128], F32, tag="sif", name=f"sif{step}")
                nc.scalar.activation(sif[:, :], gsum[:, 0:128], AF.Sigmoid)
                so = ew.tile([128, 64], F32, tag="so", name=f"so{step}")
                nc.scalar.activation(so[:, :], gsum[:, 192:256], AF.Sigmoid)
                t1 = ew.tile([128, 64], F32, tag="t1", name=f"t1_{step}")
                nc.vector.tensor_mul(t1[:, :], sif[:, 0:64], tg[:, :])
                cn = stpool.tile([128, 64], F32, tag="c_st", name=f"c_st{step}")
                nc.vector.tensor_mul(cn[:, :], sif[:, 64:128], c_prev[:, :])
                nc.vector.tensor_add(cn[:, :], cn[:, :], t1[:, :])
                cb = ew.tile([128, 64], BF, tag="cb", name=f"cb{step}")
                nc.vector.tensor_copy(cb[:, :], cn[:, :])
                nc.vector.transpose(y[:, 64:128], cb[:, :])
                tc_ = ew.tile([128, 64], F32, tag="tc", name=f"tc{step}")
                nc.scalar.activation(tc_[:, :], cn[:, :], AF.Tanh)
                hb = ew.tile([128, 64], BF, tag="hb", name=f"hb{step}")
                nc.vector.tensor_mul(hb[:, :], so[:, :], tc_[:, :])
                nc.vector.transpose(y[:, 0:64], hb[:, :])
                return cn

            def emit_gru_gi(step, cT_st):
                """gi matmul (+bias via ones-row) — depends only on AG_hc."""
                pgi = psg.tile([128, 192], F32, tag="ps_gi", name=f"ps_gi{step}")
                for k in range(KT):
                    for j in range(4):
                        nc.tensor.matmul(
                            pgi[32 * j:32 * j + 32, :],
                            cT_st[:, k * B:(k + 1) * B],
                            wgi_sb[:, k * GG + j * 192:k * GG + j * 192 + 192],
                            start=(k == 0), stop=False,
                            tile_position=(0, 32 * j),
                        )
                for j in range(4):
                    nc.tensor.matmul(
                        pgi[32 * j:32 * j + 32, :],
                        ones_sb[:, :],
                        brgi_sb[:, j * 192:(j + 1) * 192],
                        start=False, stop=(j == 3),
                        tile_position=(0, 32 * j),
                    )
                gi_sb = ew.tile([128, 192], F32, tag="gi_sb",
                                name=f"gi_sb{step}")
                nc.vector.tensor_copy(gi_sb[:, :], pgi[:, :])
                return gi_sb

            def emit_gru_gh(step, pgi, hg_prev):
                """gh matmul + elementwise tail (the latency-critical part)."""
                pgh = psg.tile([128, 192], F32, tag="ps_gh", name=f"ps_gh{step}")
                for k in range(KT):
                    for j in range(4):
                        nc.tensor.matmul(
                            pgh[32 * j:32 * j + 32, :],
                            hist_slot_k(step - 1, k),
                            wgh_sb[:, k * GG + j * 192:k * GG + j * 192 + 192],
                            start=(k == 0), stop=False,
                            tile_position=(0, 32 * j),
                        )
                for j in range(4):
                    nc.tensor.matmul(
                        pgh[32 * j:32 * j + 32, :],
                        ones_sb[:, :],
                        brgh_sb[:, j * 192:(j + 1) * 192],
                        start=False, stop=(j == 3),
                        tile_position=(0, 32 * j),
                    )
                with tc.high_priority():
                    trz = ew.tile([128, 128], F32, tag="trz", name=f"trz{step}")
                    nc.vector.tensor_add(trz[:, :], pgi[:, 0:128],
                                         pgh[:, 0:128])
                    srz = ew.tile([128, 128], F32, tag="srz", name=f"srz{step}")
                    nc.scalar.activation(srz[:, :], trz[:, :], AF.Sigmoid)
                    x1 = ew.tile([128, 64], F32, tag="x1", name=f"x1_{step}")
                    nc.vector.tensor_mul(x1[:, :], srz[:, 0:64],
                                         pgh[:, 128:192])
                    nc.vector.tensor_add(x1[:, :], x1[:, :], pgi[:, 128:192])
                    n = ew.tile([128, 64], F32, tag="n", name=f"n{step}")
                    n_inst = nc.scalar.activation(n[:, :], x1[:, :], AF.Tanh)
                    d = ew.tile([128, 64], F32, tag="d", name=f"d{step}")
                    nc.vector.tensor_sub(d[:, :], hg_prev[:, :], n[:, :])
                    nc.vector.tensor_mul(d[:, :], srz[:, 64:128], d[:, :])
                    hgn = stpool.tile([128, 64], F32, tag="hg_st",
                                      name=f"hg_st{step}")
                    nc.vector.tensor_add(hgn[:, :], n[:, :], d[:, :])
                    hgb = ew.tile([128, 64], BF, tag="hgb", name=f"hgb{step}")
                    nc.gpsimd.tensor_copy(hgb[:, :], hgn[:, :])
                    y_hg = ew.tile([128, 64], BF, tag="y_hg",
                                   name=f"y_hg{step}")
                    nc.vector.transpose(y_hg[:, :], hgb[:, :])
                return hgn, y_hg, n_inst

            def emit_bounce_out(bounce, sec, y, engine):
                # bounce stores the y tile verbatim: [r, 32*c2+b]
                return engine.dma_start(bounce[:, :], y[:, :])

            def emit_bounce_hc(bounce, y, engine):
                # bounce stores the y tile verbatim: [r, hc*64+32*c2+b]
                return engine.dma_start(bounce[:, :], y[:, :])

            def emit_gather_in(gath, nsec, sec, dst2, eng0, eng1):
                g5 = gath.ap().rearrange(
                    "(rank r) (s half b) -> s half r rank b", rank=NC, s=nsec,
                    half=2)
                i0 = eng0.dma_start(dst2(0), g5[sec][0])
                i1 = eng1.dma_start(dst2(1), g5[sec][1])
                return i0, i1

            pend_dec = {}

            def emit_decode_mm(c, vt, ncols):
                h = hists[c % 2]
                pd = psd.tile([128, 256], F32, tag="ps_d", name=f"ps_d{c}_{vt}")
                for k in range(KT):
                    nc.tensor.matmul(
                        pd[0:VTW, 0:ncols],
                        linw_sb[:, k * VS + vt * VTW:k * VS + (vt + 1) * VTW],
                        h[:, k * NHIST * B:k * NHIST * B + ncols],
                        start=(k == 0), stop=(k == KT - 1),
                    )
                pend_dec[(c, vt)] = (pd, ncols)

            def emit_decode_fin(c, vt):
                pd, ncols = pend_dec.pop((c, vt))
                stg = ew.tile([128, 256], F32, tag="stg", name=f"stg{c}_{vt}")
                nc.scalar.activation(stg[0:VTW, 0:ncols], pd[0:VTW, 0:ncols],
                                     AF.Identity, bias=linb_sb[:, vt:vt + 1])
                nc.gpsimd.dma_start(
                    out[vt * VTW:(vt + 1) * VTW,
                        c * NHIST * B:c * NHIST * B + ncols],
                    stg[0:VTW, 0:ncols])

            def emit_decode_vt(c, vt, ncols):
                emit_decode_mm(c, vt, ncols)
                emit_decode_fin(c, vt)

            ag_rg = [list(range(NC))]

            def emit_ag(bounce, gname, ncols, engine=None):
                gath = nc.dram_tensor(gname, [128 * NC, ncols], BF,
                                      addr_space="Shared")
                nc.gpsimd.collective_compute(
                    "AllGather", mybir.AluOpType.bypass,
                    replica_groups=ag_rg,
                    ins=[bounce.ap().opt()], outs=[gath.ap().opt()],
                )
                return gath

            # decode schedule: (chunk c, vocab tile vt) emitted at loop step
            # s = 8c + 9 + vt so it never waits on this step's hist gather
            dec_done = set()

            def emit_decode_for_step(s):
                for vt in range(VT):
                    rem = s - NHIST - 1 - vt
                    if rem >= 0 and rem % NHIST == 0:
                        emit_decode_mm(rem // NHIST, vt, NHIST * B)
                        dec_done.add((rem // NHIST, vt))

            def emit_decode_fin_for_step(s):
                for vt in range(VT):
                    rem = s - NHIST - 1 - vt
                    if rem >= 0 and rem % NHIST == 0:
                        emit_decode_fin(rem // NHIST, vt)

            # ---- prologue: LSTM(0) from features ----
            y0 = ew.tile([128, 128], BF, tag="y", name="y0")
            c_st = emit_lstm(0, lambda k: hT_init[:, k * B:(k + 1) * B],
                             feat_blk, y0)
            hg_st = feat_blk
            bounce0 = nc.dram_tensor("bounce0", [128, 128], BF)
            emit_bounce_hc(bounce0, y0, nc.sync)
            gaths_hc = [emit_ag(bounce0, "gathc0", 128)]
            bounceg0 = nc.dram_tensor("bounceg0", [128, 64], BF)
            nc.sync.dma_start(bounceg0[:, :], featsh_in[:, :])
            gaths_hg = [emit_ag(bounceg0, "gathg0", 64)]

            # ---- main loop ----
            # prev-iteration handles for static queue-order pinning
            prev_n = None        # GRU tail n ACT of iter s-1 (scalar)
            prev_hgb = None      # hg-bounce DMA of iter s-1 (sync)
            for s in range(ts):
                ghc = gaths_hc[s]
                # gather h(s) for LSTM(s+1): halves on sync+scalar (HW DGE)
                if s < ts - 1:
                    hT_st = spool.tile([128, KT * B], BF, tag="hT",
                                       name=f"hT{s}")

                    def hT_half(half, _t=hT_st):
                        return _t[:, :].rearrange(
                            "r (k8 k2 b) -> k2 r k8 b", k8=NC, k2=2)[half]
                    h0, h1 = emit_gather_in(ghc, 2, 0, hT_half,
                                            nc.sync, nc.scalar)
                    if prev_n is not None:
                        tile.add_dep_helper(h1.ins, prev_n.ins, sync=True,
                                            reason="scalar q order")
                # gather c(s) for GRU(s): half0 gpsimd, half1 scalar
                cT_st = spool.tile([128, KT * B], BF, tag="cT", name=f"cT{s}")

                def cT_half(half, _t=cT_st):
                    return _t[:, :].rearrange(
                        "r (k8 k2 b) -> k2 r k8 b", k8=NC, k2=2)[half]
                c0, c1 = emit_gather_in(ghc, 2, 1, cT_half,
                                        nc.gpsimd, nc.scalar)
                if prev_n is not None:
                    tile.add_dep_helper(c1.ins, prev_n.ins, sync=True,
                                        reason="scalar q order")

                if s < ts - 1:
                    # LSTM(s+1) first on PE; its AG issues during GRU(s)
                    y = ew.tile([128, 128], BF, tag="y", name=f"y{s + 1}")
                    c_st = emit_lstm(
                        s + 1, lambda k: hT_st[:, k * B:(k + 1) * B], c_st, y)
                    with tc.high_priority():
                        bounce = nc.dram_tensor(f"bounce{s + 1}",
                                                [128, 128], BF)
                        emit_bounce_hc(bounce, y, nc.sync)
                        gaths_hc.append(
                            emit_ag(bounce, f"bgathc{s + 1}", 128))

                # gi matmul right after LSTM: only needs AG_hc(s)
                pgi = emit_gru_gi(s, cT_st)

                # gather hg(s-1) hist slot (from AG_hg(s)): sync+scalar HW
                emit_gather_in(gaths_hg[s], 1, 0,
                               lambda half: hist_slot_half(s - 1, half),
                               nc.gpsimd, nc.sync)

                # decode fills the PE while gh waits on the hist gather
                emit_decode_for_step(s)

                hg_st, y_hg, prev_n = emit_gru_gh(s, pgi, hg_st)
                with tc.high_priority():
                    bg = nc.dram_tensor(f"bounceg{s + 1}", [128, 64], BF)
                    prev_hgb = emit_bounce_out(bg, 0, y_hg, nc.scalar)
                    gaths_hg.append(emit_ag(bg, f"bgathg{s + 1}", 64))
                emit_decode_fin_for_step(s)

            # ---- epilogue: last hg, leftover decode ----
            mlast = ts - 1
            emit_gather_in(gaths_hg[ts], 1, 0,
                           lambda half: hist_slot_half(mlast, half),
                           nc.gpsimd, nc.sync)
            nfull = ts // NHIST
            nchunk = nfull + (1 if ts > nfull * NHIST else 0)
            for c in range(nchunk):
                ncols = NHIST * B if c < nfull else (ts - nfull * NHIST) * B
                for vt in range(VT):
                    if (c, vt) not in dec_done:
                        emit_decode_vt(c, vt, ncols)

    nc.compile()
    return nc


def _gate_rows(core, ngates):
    """Global weight-row indices for this core's gate shard, in column order
    (jp, kappa, c2, i) with hidden-local l = 128*c2 + 32*jp + i."""
    jp = np.arange(4)[:, None, None, None]
    kappa = np.arange(ngates)[None, :, None, None]
    c2 = np.arange(2)[None, None, :, None]
    i = np.arange(32)[None, None, None, :]
    rows = kappa * H + core * HS + 128 * c2 + 32 * jp + i
    return rows.reshape(-1)


def _prep_inputs(features, captions, emb, lstm_Wih, lstm_bih, lstm_Whh,
                 lstm_bhh, gru_Wih, gru_bih, gru_Whh, gru_bhh, lin_W, lin_b,
                 ts=TS):
    f32 = np.float32
    features = np.asarray(features, f32)
    captions = np.asarray(captions)
    emb = np.asarray(emb, f32)
    lstm_Wih = np.asarray(lstm_Wih, f32); lstm_bih = np.asarray(lstm_bih, f32)
    lstm_Whh = np.asarray(lstm_Whh, f32); lstm_bhh = np.asarray(lstm_bhh, f32)
    gru_Wih = np.asarray(gru_Wih, f32); gru_bih = np.asarray(gru_bih, f32)
    gru_Whh = np.asarray(gru_Whh, f32); gru_bhh = np.asarray(gru_bhh, f32)
    lin_W = np.asarray(lin_W, f32); lin_b = np.asarray(lin_b, f32)

    xs = emb[captions[:, :ts]]                      # [B, ts, E]
    xs_aug = np.ones((E + 1, ts * B), f32)
    xs_aug[:E, :] = xs.transpose(2, 1, 0).reshape(E, ts * B)

    featT = features.T.copy()                       # [H, B]
    ones = np.ones((1, B), f32)

    in_maps = []
    for core in range(NC):
        rl = _gate_rows(core, 4)
        rg = _gate_rows(core, 3)
        wl = lstm_Whh[rl, :].T
        wih = np.concatenate(
            [lstm_Wih[rl, :].T,
             (lstm_bih[rl] + lstm_bhh[rl])[None, :]], axis=0)
        wgi = gru_Wih[rg, :].T
        brgi = gru_bih[rg].reshape(1, GG)
        wgh = gru_Whh[rg, :].T
        brgh = gru_bhh[rg].reshape(1, GG)
        linw = lin_W[core * VS:(core + 1) * VS, :].T
        linb = lin_b[core * VS:(core + 1) * VS].reshape(VT, VTW).T.copy()
        fsh = features[:, core * HS:(core + 1) * HS].reshape(B, 2, 4, 32)
        feat_sh = fsh.transpose(2, 3, 1, 0).reshape(128, 64).copy()
        # feat_blk [32*jp+b, 32*c2+i] = features[b, core*HS + 128*c2+32*jp+i]
        fb = features[:, core * HS:(core + 1) * HS].reshape(B, 2, 4, 32)
        feat_blk = fb.transpose(2, 0, 1, 3).reshape(128, 64).copy()

        bf = BF16
        in_maps.append({
            "wl": wl.astype(bf), "wih": wih.astype(bf),
            "wgi": wgi.astype(bf), "brgi": brgi.astype(bf),
            "wgh": wgh.astype(bf), "brgh": brgh.astype(bf),
            "linw": linw.astype(bf),
            "linb": linb.astype(f32),
            "xs_aug": xs_aug.astype(bf),
            "featT": featT.astype(bf),
            "feat_sh": feat_sh.astype(bf),
            "feat_blk": feat_blk.astype(f32),
            "ones": ones.astype(bf),
        })
    return in_maps


def kernel(**inputs):
    ts = TS
    if ts not in _BUILD_CACHE:
        _BUILD_CACHE[ts] = _build(ts)
    nc = _BUILD_CACHE[ts]
    in_maps = _prep_inputs(**inputs, ts=ts)
    res = bass_utils.run_bass_kernel_spmd(nc, in_maps,
                                          core_ids=list(range(NC)))
    full = np.empty((B, ts, V), np.float32)
    for core in range(NC):
        o = res.results[core]["out"]                 # [VS, ts*B]
        full[:, :, core * VS:(core + 1) * VS] = (
            o.reshape(VS, ts, B).transpose(2, 1, 0))
    return full



# revision 19
# speedup vs baseline: 1.2461x; 1.1003x over previous
"""Trainium2 Bass kernel for nn_Decoder (LSTMCell -> GRUCell -> Linear decode).

Strategy (8 NeuronCores, one chip):
  - Hidden dim H=2048 sharded 8 ways (256/core). Each core holds the weight
    rows for its hidden slice of the LSTM/GRU gates in SBUF (bf16), computes
    its gate shard with batch-major col-tiled matmuls (stationary = h-major
    state tiles [128,32], moving = weight columns), applies the elementwise
    cell updates in fp32, stream-transposes its new state shard to h-major
    bf16 and exchanges it via two pipelined AllGathers per step: AG_hc
    carries [h(s+1), c(s+1)] (issued right after the LSTM, while the GRU
    still computes) and AG_hg carries [hg(s)].
  - Hidden-index layout l = 128*c2 + 32*jp + i is chosen so the 32x32-block
    StreamTranspose of the [128,64] state tile directly yields the h-major
    shard, and every DRAM exchange is a single strided DMA.
  - The vocab-sharded linear decode (1000 rows/core) consumes a double-
    buffered history of gathered hg, one vocab tile per step, hiding under
    the collective latency and keeping TensorE warm.
  - kernel(**inputs) takes FULL inputs, shards on host, runs the SPMD NEFF
    on cores 0-7 via run_bass_kernel_spmd, reassembles the FULL output.
"""
import os
import sys

import numpy as np

for _p in ("/root/.axon_site", "/root/.axon_site/_ro/trn_rl_repo",
           "/root/.axon_site/_ro/pypackages", "/opt/trn_rl_repo"):
    if os.path.isdir(_p) and _p not in sys.path:
        sys.path.append(_p)

import concourse.bacc as bacc
import concourse.bass as bass
import concourse.mybir as mybir
import concourse.tile as tile
from concourse import bass_utils

import ml_dtypes

BF16 = ml_dtypes.bfloat16
F32 = mybir.dt.float32
BF = mybir.dt.bfloat16
AF = mybir.ActivationFunctionType

NC = 8          # cores
B = 32          # batch
T = 40          # caption length
TS = T - 1      # recurrent steps
V = 8000
E = 50
H = 2048
HS = H // NC    # 256 hidden per core
VS = V // NC    # 1000 vocab per core
KT = H // 128   # 16 contraction tiles
VT = 8          # vocab tiles per core
VTW = VS // VT  # 125 cols per vocab tile
GL = 4 * HS     # 1024 lstm gate cols per core
GG = 3 * HS     # 768 gru gate cols per matmul per core
NHIST = 8       # decode chunk length

_BUILD_CACHE = {}


def _build(ts=TS):
    nc = bacc.Bacc("TRN2", target_bir_lowering=False, debug=False,
                   enable_asserts=True, num_devices=NC)

    # ---- external I/O (per core) ----
    wl_in = nc.dram_tensor("wl", [H, GL], BF, kind="ExternalInput")
    wih_in = nc.dram_tensor("wih", [E + 1, GL], BF, kind="ExternalInput")
    wgi_in = nc.dram_tensor("wgi", [H, GG], BF, kind="ExternalInput")
    brgi_in = nc.dram_tensor("brgi", [1, GG], BF, kind="ExternalInput")
    wgh_in = nc.dram_tensor("wgh", [H, GG], BF, kind="ExternalInput")
    brgh_in = nc.dram_tensor("brgh", [1, GG], BF, kind="ExternalInput")
    linw_in = nc.dram_tensor("linw", [H, VS], BF, kind="ExternalInput")
    linb_in = nc.dram_tensor("linb", [VTW, VT], F32, kind="ExternalInput")
    xs_in = nc.dram_tensor("xs_aug", [E + 1, ts * B], BF, kind="ExternalInput")
    featT_in = nc.dram_tensor("featT", [H, B], BF, kind="ExternalInput")
    featsh_in = nc.dram_tensor("feat_sh", [128, 64], BF, kind="ExternalInput")
    featblk_in = nc.dram_tensor("feat_blk", [128, 64], F32, kind="ExternalInput")
    ones_in = nc.dram_tensor("ones", [1, B], BF, kind="ExternalInput")
    out = nc.dram_tensor("out", [VS, ts * B], F32, kind="ExternalOutput")

    with tile.TileContext(nc) as tc:
        with (
            tc.tile_pool(name="const", bufs=1) as cpool,
            tc.tile_pool(name="stat", bufs=3) as spool,
            tc.tile_pool(name="state", bufs=2) as stpool,
            tc.tile_pool(name="ew", bufs=3) as ew,
            tc.tile_pool(name="psl", bufs=2, space="PSUM") as psl,
            tc.tile_pool(name="psg", bufs=2, space="PSUM") as psg,
            tc.tile_pool(name="psd", bufs=2, space="PSUM") as psd,
        ):
            # warmup collective: absorbs the ~12us cold-start of the CC
            # stream while the weight DMAs stream in
            ones_sb = cpool.tile([1, B], BF)
            nc.sync.dma_start(ones_sb[:, :], ones_in[:, :])
            warm_in = nc.dram_tensor("warmi", [1, B], BF)
            nc.sync.dma_start(warm_in[:, :], ones_sb[:, :])
            warm_out = nc.dram_tensor("warmg", [NC, B], BF,
                                      addr_space="Shared")
            nc.gpsimd.collective_compute(
                "AllGather", mybir.AluOpType.bypass,
                replica_groups=[list(range(NC))],
                ins=[warm_in.ap().opt()], outs=[warm_out.ap().opt()])

            # ---- load weights / constants into SBUF (single strided DMAs) --
            wl_sb = cpool.tile([128, KT * GL], BF)
            nc.sync.dma_start(
                wl_sb[:, :].rearrange("r (k c) -> r k c", k=KT),
                wl_in[:, :].rearrange("(k r) c -> r k c", k=KT))
            wgi_sb = cpool.tile([128, KT * GG], BF)
            nc.sync.dma_start(
                wgi_sb[:, :].rearrange("r (k c) -> r k c", k=KT),
                wgi_in[:, :].rearrange("(k r) c -> r k c", k=KT))
            wgh_sb = cpool.tile([128, KT * GG], BF)
            nc.sync.dma_start(
                wgh_sb[:, :].rearrange("r (k c) -> r k c", k=KT),
                wgh_in[:, :].rearrange("(k r) c -> r k c", k=KT))
            linw_sb = cpool.tile([128, KT * VS], BF)
            nc.scalar.dma_start(
                linw_sb[:, :].rearrange("r (k c) -> r k c", k=KT),
                linw_in[:, :].rearrange("(k r) c -> r k c", k=KT))
            wih_sb = cpool.tile([E + 1, GL], BF)
            nc.sync.dma_start(wih_sb[:, :], wih_in[:, :])
            brgi_sb = cpool.tile([1, GG], BF)
            nc.sync.dma_start(brgi_sb[:, :], brgi_in[:, :])
            brgh_sb = cpool.tile([1, GG], BF)
            nc.sync.dma_start(brgh_sb[:, :], brgh_in[:, :])
            linb_sb = cpool.tile([VTW, VT], F32)
            nc.scalar.dma_start(linb_sb[:, :], linb_in[:, :])
            xs_sb = cpool.tile([E + 1, ts * B], BF)
            nc.sync.dma_start(xs_sb[:, :], xs_in[:, :])
            feat_blk = cpool.tile([128, 64], F32)
            nc.sync.dma_start(feat_blk[:, :], featblk_in[:, :])
            hT_init = cpool.tile([128, KT * B], BF)
            nc.sync.dma_start(
                hT_init[:, :].rearrange("r (k b) -> r k b", k=KT),
                featT_in[:, :].rearrange("(k r) b -> r k b", k=KT))
            # double-buffered gathered-hg history (h-major, bf16)
            hists = [cpool.tile([128, KT * NHIST * B], BF, name=f"hist{p}")
                     for p in range(2)]

            def hist_slot_dst(m):
                """DMA-dst AP [r, rank, 64] for hg(m)'s history slot."""
                h = hists[(m // NHIST) % 2]
                return h[:, :].rearrange(
                    "r (t k8 f) -> t r k8 f", t=NHIST, k8=NC)[m % NHIST]

            def hist_slot_k(m, k):
                """Stationary AP [128, B] for hg(m) k-tile k=(rank, c2)."""
                h = hists[(m // NHIST) % 2]
                off = (m % NHIST) * NC * 64 + (k // 2) * 64 + (k % 2) * B
                return h[:, off:off + B]

            def hist_mov(p, k, nt):
                """Decode moving AP [128, nt, B]: chunk buffer p, k-tile k,
                first nt slots."""
                return hists[p][:, 0:nt * NC * 64].rearrange(
                    "r (t k8 c2 b) -> k8 c2 r t b", t=nt, k8=NC,
                    c2=2)[k // 2][k % 2]

            def emit_lstm(step, hT_ap, c_prev, y):
                gsum = psl.tile([128, 256], F32, tag="ps_l", name=f"ps_l{step}")
                for k in range(KT):
                    for j in range(4):
                        nc.tensor.matmul(
                            gsum[32 * j:32 * j + 32, :],
                            hT_ap(k),
                            wl_sb[:, k * GL + j * 256:k * GL + j * 256 + 256],
                            start=(k == 0), stop=False,
                            tile_position=(0, 32 * j),
                        )
                for j in range(4):
                    nc.tensor.matmul(
                        gsum[32 * j:32 * j + 32, :],
                        xs_sb[:, step * B:(step + 1) * B],
                        wih_sb[:, j * 256:j * 256 + 256],
                        start=False, stop=True,
                        tile_position=(0, 32 * j),
                    )
                tc2 = tc
                with tc2.high_priority():
                    return emit_lstm_tail(step, gsum, c_prev, y)

            def emit_lstm_tail(step, gsum, c_prev, y):
                tg = ew.tile([128, 64], F32, tag="tg", name=f"tg{step}")
                nc.scalar.activation(tg[:, :], gsum[:, 128:192], AF.Tanh)
                sif = ew.tile([128, 128], F32, tag="sif", name=f"sif{step}")
                nc.scalar.activation(sif[:, :], gsum[:, 0:128], AF.Sigmoid)
                so = ew.tile([128, 64], F32, tag="so", name=f"so{step}")
                nc.scalar.activation(so[:, :], gsum[:, 192:256], AF.Sigmoid)
                t1 = ew.tile([128, 64], F32, tag="t1", name=f"t1_{step}")
                nc.vector.tensor_mul(t1[:, :], sif[:, 0:64], tg[:, :])
                cn = stpool.tile([128, 64], F32, tag="c_st", name=f"c_st{step}")
                nc.vector.tensor_mul(cn[:, :], sif[:, 64:128], c_prev[:, :])
                nc.vector.tensor_add(cn[:, :], cn[:, :], t1[:, :])
                cb = ew.tile([128, 64], BF, tag="cb", name=f"cb{step}")
                nc.vector.tensor_copy(cb[:, :], cn[:, :])
                nc.vector.transpose(y[:, 64:128], cb[:, :])
                tc_ = ew.tile([128, 64], F32, tag="tc", name=f"tc{step}")
                nc.scalar.activation(tc_[:, :], cn[:, :], AF.Tanh)
                hb = ew.tile([128, 64], BF, tag="hb", name=f"hb{step}")
                nc.vector.tensor_mul(hb[:, :], so[:, :], tc_[:, :])
                nc.vector.transpose(y[:, 0:64], hb[:, :])
                return cn

            def emit_gru_gi(step, cT_ap):
                """gi matmul (+bias via ones-row) — depends only on AG_hc."""
                pgi = psg.tile([128, 192], F32, tag="ps_gi", name=f"ps_gi{step}")
                for k in range(KT):
                    for j in range(4):
                        nc.tensor.matmul(
                            pgi[32 * j:32 * j + 32, :],
                            cT_ap(k),
                            wgi_sb[:, k * GG + j * 192:k * GG + j * 192 + 192],
                            start=(k == 0), stop=False,
                            tile_position=(0, 32 * j),
                        )
                for j in range(4):
                    nc.tensor.matmul(
                        pgi[32 * j:32 * j + 32, :],
                        ones_sb[:, :],
                        brgi_sb[:, j * 192:(j + 1) * 192],
                        start=False, stop=(j == 3),
                        tile_position=(0, 32 * j),
                    )
                gi_sb = ew.tile([128, 192], F32, tag="gi_sb",
                                name=f"gi_sb{step}")
                nc.vector.tensor_copy(gi_sb[:, :], pgi[:, :])
                return gi_sb

            def emit_gru_gh(step, pgi, hg_prev):
                """gh matmul + elementwise tail (the latency-critical part)."""
                pgh = psg.tile([128, 192], F32, tag="ps_gh", name=f"ps_gh{step}")
                for k in range(KT):
                    for j in range(4):
                        nc.tensor.matmul(
                            pgh[32 * j:32 * j + 32, :],
                            hist_slot_k(step - 1, k),
                            wgh_sb[:, k * GG + j * 192:k * GG + j * 192 + 192],
                            start=(k == 0), stop=False,
                            tile_position=(0, 32 * j),
                        )
                for j in range(4):
                    nc.tensor.matmul(
                        pgh[32 * j:32 * j + 32, :],
                        ones_sb[:, :],
                        brgh_sb[:, j * 192:(j + 1) * 192],
                        start=False, stop=(j == 3),
                        tile_position=(0, 32 * j),
                    )
                with tc.high_priority():
                    trz = ew.tile([128, 128], F32, tag="trz", name=f"trz{step}")
                    nc.vector.tensor_add(trz[:, :], pgi[:, 0:128],
                                         pgh[:, 0:128])
                    srz = ew.tile([128, 128], F32, tag="srz", name=f"srz{step}")
                    nc.scalar.activation(srz[:, :], trz[:, :], AF.Sigmoid)
                    x1 = ew.tile([128, 64], F32, tag="x1", name=f"x1_{step}")
                    nc.vector.tensor_mul(x1[:, :], srz[:, 0:64],
                                         pgh[:, 128:192])
                    nc.vector.tensor_add(x1[:, :], x1[:, :], pgi[:, 128:192])
                    n = ew.tile([128, 64], F32, tag="n", name=f"n{step}")
                    n_inst = nc.scalar.activation(n[:, :], x1[:, :], AF.Tanh)
                    d = ew.tile([128, 64], F32, tag="d", name=f"d{step}")
                    nc.vector.tensor_sub(d[:, :], hg_prev[:, :], n[:, :])
                    nc.vector.tensor_mul(d[:, :], srz[:, 64:128], d[:, :])
                    hgn = stpool.tile([128, 64], F32, tag="hg_st",
                                      name=f"hg_st{step}")
                    nc.vector.tensor_add(hgn[:, :], n[:, :], d[:, :])
                    hgb = ew.tile([128, 64], BF, tag="hgb", name=f"hgb{step}")
                    nc.gpsimd.tensor_copy(hgb[:, :], hgn[:, :])
                    y_hg = ew.tile([128, 64], BF, tag="y_hg",
                                   name=f"y_hg{step}")
                    nc.vector.transpose(y_hg[:, :], hgb[:, :])
                return hgn, y_hg, n_inst

            def emit_bounce_out(bounce, sec, y, engine):
                # bounce stores the y tile verbatim: [r, 32*c2+b]
                return engine.dma_start(bounce[:, :], y[:, :])

            def emit_bounce_hc(bounce, y, engine):
                # bounce stores the y tile verbatim: [r, hc*64+32*c2+b]
                return engine.dma_start(bounce[:, :], y[:, :])

            def emit_gather_in(gath, dst, engine):
                g = gath.ap().rearrange("(rank r) f -> r rank f", rank=NC)
                return engine.dma_start(dst, g)

            pend_dec = {}

            def emit_decode_mm(c, vt, ncols):
                pd = psd.tile([128, 256], F32, tag="ps_d", name=f"ps_d{c}_{vt}")
                nt = ncols // B
                for k in range(KT):
                    nc.tensor.matmul(
                        pd[0:VTW, 0:ncols],
                        linw_sb[:, k * VS + vt * VTW:k * VS + (vt + 1) * VTW],
                        hist_mov(c % 2, k, nt),
                        start=(k == 0), stop=(k == KT - 1),
                    )
                pend_dec[(c, vt)] = (pd, ncols)

            def emit_decode_fin(c, vt):
                pd, ncols = pend_dec.pop((c, vt))
                stg = ew.tile([128, 256], F32, tag="stg", name=f"stg{c}_{vt}")
                nc.scalar.activation(stg[0:VTW, 0:ncols], pd[0:VTW, 0:ncols],
                                     AF.Identity, bias=linb_sb[:, vt:vt + 1])
                nc.gpsimd.dma_start(
                    out[vt * VTW:(vt + 1) * VTW,
                        c * NHIST * B:c * NHIST * B + ncols],
                    stg[0:VTW, 0:ncols])

            def emit_decode_vt(c, vt, ncols):
                emit_decode_mm(c, vt, ncols)
                emit_decode_fin(c, vt)

            ag_rg = [list(range(NC))]

            def emit_ag(bounce, gname, ncols, engine=None):
                gath = nc.dram_tensor(gname, [128 * NC, ncols], BF,
                                      addr_space="Shared")
                nc.gpsimd.collective_compute(
                    "AllGather", mybir.AluOpType.bypass,
                    replica_groups=ag_rg,
                    ins=[bounce.ap().opt()], outs=[gath.ap().opt()],
                )
                return gath

            # decode schedule: (chunk c, vocab tile vt) emitted at loop step
            # s = 8c + 9 + vt so it never waits on this step's hist gather
            dec_done = set()

            def emit_decode_for_step(s):
                for vt in range(VT):
                    rem = s - NHIST - 1 - vt
                    if rem >= 0 and rem % NHIST == 0:
                        emit_decode_mm(rem // NHIST, vt, NHIST * B)
                        dec_done.add((rem // NHIST, vt))

            def emit_decode_fin_for_step(s):
                for vt in range(VT):
                    rem = s - NHIST - 1 - vt
                    if rem >= 0 and rem % NHIST == 0:
                        emit_decode_fin(rem // NHIST, vt)

            # ---- prologue: LSTM(0) from features ----
            y0 = ew.tile([128, 128], BF, tag="y", name="y0")
            c_st = emit_lstm(0, lambda k: hT_init[:, k * B:(k + 1) * B],
                             feat_blk, y0)
            hg_st = feat_blk
            bounce0 = nc.dram_tensor("bounce0", [128, 128], BF)
            emit_bounce_hc(bounce0, y0, nc.sync)
            gaths_hc = [emit_ag(bounce0, "gathc0", 128)]
            bounceg0 = nc.dram_tensor("bounceg0", [128, 64], BF)
            nc.sync.dma_start(bounceg0[:, :], featsh_in[:, :])
            gaths_hg = [emit_ag(bounceg0, "gathg0", 64)]

            # ---- main loop ----
            for s in range(ts):
                ghc = gaths_hc[s]
                # gather h(s)+c(s) in ONE contiguous-run DMA; SBUF layout
                # mirrors the gathered DRAM layout [r, rank, 128]
                hcT = spool.tile([128, NC * 128], BF, tag="hcT",
                                 name=f"hcT{s}")
                emit_gather_in(
                    ghc,
                    hcT[:, :].rearrange("r (k8 f) -> r k8 f", k8=NC),
                    nc.sync)

                def hT_ap(k, _t=hcT):
                    off = (k // 2) * 128 + (k % 2) * B
                    return _t[:, off:off + B]

                def cT_ap(k, _t=hcT):
                    off = (k // 2) * 128 + 64 + (k % 2) * B
                    return _t[:, off:off + B]

                if s < ts - 1:
                    # LSTM(s+1) first on PE; its AG issues during GRU(s)
                    y = ew.tile([128, 128], BF, tag="y", name=f"y{s + 1}")
                    c_st = emit_lstm(s + 1, hT_ap, c_st, y)
                    with tc.high_priority():
                        bounce = nc.dram_tensor(f"bounce{s + 1}",
                                                [128, 128], BF)
                        emit_bounce_hc(bounce, y, nc.sync)
                        gaths_hc.append(
                            emit_ag(bounce, f"bgathc{s + 1}", 128))

                # gi matmul right after LSTM: only needs AG_hc(s)
                pgi = emit_gru_gi(s, cT_ap)

                # gather hg(s-1) hist slot in ONE DMA (sync HW DGE)
                emit_gather_in(gaths_hg[s], hist_slot_dst(s - 1), nc.sync)

                # decode fills the PE while gh waits on the hist gather
                emit_decode_for_step(s)

                hg_st, y_hg, _n = emit_gru_gh(s, pgi, hg_st)
                with tc.high_priority():
                    bg = nc.dram_tensor(f"bounceg{s + 1}", [128, 64], BF)
                    emit_bounce_out(bg, 0, y_hg, nc.scalar)
                    gaths_hg.append(emit_ag(bg, f"bgathg{s + 1}", 64))
                emit_decode_fin_for_step(s)

            # ---- epilogue: last hg, leftover decode ----
            mlast = ts - 1
            emit_gather_in(gaths_hg[ts], hist_slot_dst(mlast), nc.sync)
            nfull = ts // NHIST
            nchunk = nfull + (1 if ts > nfull * NHIST else 0)
            for c in range(nchunk):
                ncols = NHIST * B if c < nfull else (ts - nfull * NHIST) * B
                for vt in range(VT):
                    if (c, vt) not in dec_done:
                        emit_decode_vt(c, vt, ncols)

    nc.compile()
    return nc


def _gate_rows(core, ngates):
    """Global weight-row indices for this core's gate shard, in column order
    (jp, kappa, c2, i) with hidden-local l = 128*c2 + 32*jp + i."""
    jp = np.arange(4)[:, None, None, None]
    kappa = np.arange(ngates)[None, :, None, None]
    c2 = np.arange(2)[None, None, :, None]
    i = np.arange(32)[None, None, None, :]
    rows = kappa * H + core * HS + 128 * c2 + 32 * jp + i
    return rows.reshape(-1)


def _prep_inputs(features, captions, emb, lstm_Wih, lstm_bih, lstm_Whh,
                 lstm_bhh, gru_Wih, gru_bih, gru_Whh, gru_bhh, lin_W, lin_b,
                 ts=TS):
    f32 = np.float32
    features = np.asarray(features, f32)
    captions = np.asarray(captions)
    emb = np.asarray(emb, f32)
    lstm_Wih = np.asarray(lstm_Wih, f32); lstm_bih = np.asarray(lstm_bih, f32)
    lstm_Whh = np.asarray(lstm_Whh, f32); lstm_bhh = np.asarray(lstm_bhh, f32)
    gru_Wih = np.asarray(gru_Wih, f32); gru_bih = np.asarray(gru_bih, f32)
    gru_Whh = np.asarray(gru_Whh, f32); gru_bhh = np.asarray(gru_bhh, f32)
    lin_W = np.asarray(lin_W, f32); lin_b = np.asarray(lin_b, f32)

    xs = emb[captions[:, :ts]]                      # [B, ts, E]
    xs_aug = np.ones((E + 1, ts * B), f32)
    xs_aug[:E, :] = xs.transpose(2, 1, 0).reshape(E, ts * B)

    featT = features.T.copy()                       # [H, B]
    ones = np.ones((1, B), f32)

    in_maps = []
    for core in range(NC):
        rl = _gate_rows(core, 4)
        rg = _gate_rows(core, 3)
        wl = lstm_Whh[rl, :].T
        wih = np.concatenate(
            [lstm_Wih[rl, :].T,
             (lstm_bih[rl] + lstm_bhh[rl])[None, :]], axis=0)
        wgi = gru_Wih[rg, :].T
        brgi = gru_bih[rg].reshape(1, GG)
        wgh = gru_Whh[rg, :].T
        brgh = gru_bhh[rg].reshape(1, GG)
        linw = lin_W[core * VS:(core + 1) * VS, :].T
        linb = lin_b[core * VS:(core + 1) * VS].reshape(VT, VTW).T.copy()
        fsh = features[:, core * HS:(core + 1) * HS].reshape(B, 2, 4, 32)
        feat_sh = fsh.transpose(2, 3, 1, 0).reshape(128, 64).copy()
        # feat_blk [32*jp+b, 32*c2+i] = features[b, core*HS + 128*c2+32*jp+i]
        fb = features[:, core * HS:(core + 1) * HS].reshape(B, 2, 4, 32)
        feat_blk = fb.transpose(2, 0, 1, 3).reshape(128, 64).copy()

        bf = BF16
        in_maps.append({
            "wl": wl.astype(bf), "wih": wih.astype(bf),
            "wgi": wgi.astype(bf), "brgi": brgi.astype(bf),
            "wgh": wgh.astype(bf), "brgh": brgh.astype(bf),
            "linw": linw.astype(bf),
            "linb": linb.astype(f32),
            "xs_aug": xs_aug.astype(bf),
            "featT": featT.astype(bf),
            "feat_sh": feat_sh.astype(bf),
            "feat_blk": feat_blk.astype(f32),
            "ones": ones.astype(bf),
        })
    return in_maps


def kernel(**inputs):
    ts = TS
    if ts not in _BUILD_CACHE:
        _BUILD_CACHE[ts] = _build(ts)
    nc = _BUILD_CACHE[ts]
    in_maps = _prep_inputs(**inputs, ts=ts)
    res = bass_utils.run_bass_kernel_spmd(nc, in_maps,
                                          core_ids=list(range(NC)))
    full = np.empty((B, ts, V), np.float32)
    for core in range(NC):
        o = res.results[core]["out"]                 # [VS, ts*B]
        full[:, :, core * VS:(core + 1) * VS] = (
            o.reshape(VS, ts, B).transpose(2, 1, 0))
    return full

